# revision 67
# baseline (speedup 1.0000x reference)
"""Trainium2 Bass kernel for nn_Attentive_VLP_LSTM.

kernel(**inputs) takes the FULL unsharded inputs (numpy) and returns the
FULL [B, T, 3] output. Batch is sharded over 8 NeuronCores (32 rows
each); each core runs a fully-unrolled T=256 recurrent Bass/Tile program.

Design notes (v3 — fp16 matmuls + software pipelining):
  - All matmuls run in fp16 (1 PE pass @ 1 cyc/row vs fp32's 2 passes @
    4 cyc/row; fp16 over bf16 for the 8x finer mantissa at equal speed).
    The distance matmul survives fp16 by recentering all positions by
    CEN=2.5 (shrinks the |led|^2-scale terms whose cancellation dominates
    the quantization error of d^2).
  - Single ACT table set (natural_log_exp_and_others): softmax uses
    exp(q.k - 0.5*ln(d^2+eps)); LayerNorm rstd = exp(-0.5*ln(var+eps));
    LSTM sigmoid/tanh built from exp + DVE reciprocal_approx_fast.
    Gate layout is (i,f,g,o): the o-gate sigmoid runs on DVE during the
    exp(-2c) ACT latency. Relu runs as a DVE max when biases are zero.
  - Cross-step software pipelining: the Whh gate matmuls for step t+1
    are issued as soon as h(t) is ready (they hide inside step t's
    DVE/ACT phase). The framework list-schedules by dependencies, so the
    rss half of the q-MLP hoists itself off the critical path.
  - PSUM accumulation-group pitfalls (hardware-verified):
    * start=True clears has_written beyond the instruction's own write
      region - a multi-region group must put start=True ONLY on its
      first matmul, or later accumulating writes silently overwrite.
    * a group whose start/stop halves are textually far apart (emitted
      across the step boundary) can lose its start flag entirely in
      compilation; keep groups contiguous in emission order.
    * DVE/custom-DVE ops must not read two PSUM operands, and
      reciprocal_approx_fast mis-reads PSUM inputs outright.
  - LEDs are sorted by frequency and laid out in 32-aligned slots per
    frequency group (matmul tile_position needs 32-aligned bases); the
    per-(batch, rss-row) softmax runs only over its own group; all 16
    slots are emitted (dummies hit KWT_e's zero columns) so no PSUM row
    feeds exp() uninitialized. Group sums + led_feat aggregation are one
    block-diagonal matmul (XW).
  - Activations are feature-major [feat, batch]; biases are folded into
    matmuls via ones-rows. Output accumulates in SBUF [3, T*32], host
    transposes. r3 broadcast PSUM shares the dist bank (disjoint in
    time) so everything fits in 8 PSUM banks.
"""

import os
import sys

import numpy as np

for _p in ("/opt/trn_rl_repo", "/root/.axon_site", "/root/.axon_site/_ro/pypackages"):
    if _p not in sys.path and os.path.isdir(_p):
        sys.path.append(_p)

import ml_dtypes

import concourse.bass as bass
import concourse.tile as tile
from concourse import bacc, mybir
from concourse.bass_utils import run_bass_kernel_spmd

# The act-table chooser assigns each activation the FIRST table set that
# contains its function, which thrashes between `natural_log` and
# `exp_and_others` (one ~1.3us reload per switch, several per step).
# Every ACT function this kernel uses (Exp, Ln, Relu) lives in
# `natural_log_exp_and_others`, so blank out every other set (order and
# indices preserved -> act_func_set_id stays correct) to get ONE load.
_KEEP_ACT_SET = "natural_log_exp_and_others"
_orig_get_act_tables = bacc.get_activation_tables


def _patched_get_act_tables(arch):
    tabs = _orig_get_act_tables(arch)
    return {name: (fns if name == _KEEP_ACT_SET else set())
            for name, fns in tabs.items()}


bacc.get_activation_tables = _patched_get_act_tables

B, T, RSS, L, FEAT, H = 256, 256, 12, 256, 8, 128
NCORES = 8
BL = B // NCORES  # 32 batch rows per core
AFT = mybir.ActivationFunctionType
ALU = mybir.AluOpType
F32 = mybir.dt.float32
LOWP = mybir.dt.float16   # fp16: 1 cyc/row on PE like bf16, 8x finer mantissa
NPLP = np.float16
if os.environ.get("KF32"):        # debug: full fp32 matmuls
    LOWP = mybir.dt.float32
    NPLP = np.float32
DIST_EPS = 1e-8
LN_EPS = 1e-5
CEN = 2.5  # room-center shift for the fp16 distance matmul

# Gate column-block order in the [128, 4*BL] gate psum: i, f, g, o
# (torch weight-row order, kept as-is). g pre-doubled for the tanh fold;
# o last so its sigmoid chain can overlap the exp(-2c) ACT latency.
_GATE_ROWS = [(0, H), (H, 2 * H), (2 * H, 3 * H), (3 * H, 4 * H)]


def _f32(x):
    return np.ascontiguousarray(np.asarray(x, np.float32))


def _lowp(x):
    return np.ascontiguousarray(np.asarray(x, NPLP))


def _host_prep(inputs):
    """All static marshalling: LED sort + 32-aligned slot layout, small
    one-time MLPs, weight folds."""
    gpf = _f32(inputs["global_led_pos_freq"])  # [L, 4]
    freq = gpf[:, 3]
    perm = np.argsort(freq, kind="stable")
    gpf_p = gpf[perm]

    relu = lambda x: np.maximum(x, np.float32(0))
    lin = lambda x, W, bb: x @ _f32(W).T + _f32(bb)

    led_feat = relu(lin(relu(lin(gpf_p, inputs["enc_W1"], inputs["enc_b1"])),
                        inputs["enc_W2"], inputs["enc_b2"]))  # [L, 8]
    keys = lin(relu(lin(np.concatenate([led_feat, gpf_p[:, :3]], 1),
                        inputs["k_W1"], inputs["k_b1"])),
               inputs["k_W2"], inputs["k_b2"])  # [L, 64]

    # --- padded slot layout: each freq group starts at a 32-boundary ---
    freq_p = gpf_p[:, 3]
    bounds = np.searchsorted(freq_p, np.arange(1, RSS + 2) - 0.5)
    slot_of_group = []   # list of (slot_base, r) 32-wide scores-MM slots
    src_list = []
    base = 0
    for r in range(RSS):
        g0, g1 = int(bounds[r]), int(bounds[r + 1])
        n = g1 - g0
        nslots = max(1, (n + 31) // 32)
        for s in range(nslots):
            slot_of_group.append((base + 32 * s, r))
        sl = -np.ones(nslots * 32, np.int64)
        sl[:n] = np.arange(g0, g1)
        src_list.append(sl)
        base += nslots * 32
    pad_src = np.concatenate(src_list)  # padded-pos -> sorted-led idx or -1
    LP = ((base + 127) // 128) * 128    # padded led count, 128-multiple
    pad_src = np.concatenate([pad_src, -np.ones(LP - base, np.int64)])
    NC = LP // 128
    real = pad_src >= 0

    def expand(arr_p, fill=0.0):
        """[L, ...] sorted-led array -> [LP, ...] padded."""
        out = np.full((LP,) + arr_p.shape[1:], fill, np.float32)
        out[real] = arr_p[pad_src[real]]
        return out

    led_feat_e = expand(led_feat)
    keys_e = expand(keys)
    pos_e = expand(gpf_p[:, :3], fill=100.0)  # dummies far away (fp16-safe)
    r_of_e = np.full(LP, 0, np.int64)
    r_of_e[real] = np.rint(freq_p[pad_src[real]] - 1.0).astype(np.int64)

    KW = 2.0 * keys_e @ _f32(inputs["q_W2"])      # [LP, 64]
    kb2 = 2.0 * keys_e @ _f32(inputs["q_b2"])     # [LP]
    KWT_e = _f32(np.concatenate([KW.T, kb2[None, :]], 0))  # [65, LP]

    # prevaux rows: 0:3 pred-CEN, 3 ones, 4:32 zero pad, 32:35 (pred-CEN)^2
    # (compute-engine writes must start at 32-aligned partitions).
    # Positions are recentered by CEN so the d^2 matmul is fp16-safe:
    # |led|^2-scale terms shrink ~4x, keeping the catastrophic-cancellation
    # error of quantized inputs well below ln()'s sensitivity.
    pos_c = pos_e - CEN
    poshT = np.zeros((35, LP), np.float32)
    poshT[0:3] = -2.0 * pos_c.T
    poshT[3] = (pos_c * pos_c).sum(1)
    poshT[32:35] = 1.0

    # XW [NC][128, 108]: cols 0:96 = agg (r*8+j), 96:108 = group sums
    XW = np.zeros((NC, 128, 108), np.float32)
    for lp in range(LP):
        if not real[lp]:
            continue
        c, lr = divmod(lp, 128)
        r = r_of_e[lp]
        XW[c, lr, r * 8:(r + 1) * 8] = led_feat_e[lp]
        XW[c, lr, 96 + r] = 1.0

    Ebc = np.zeros((RSS, 96), np.float32)
    for r in range(RSS):
        Ebc[r, r * 8:(r + 1) * 8] = 1.0

    def gate_perm_cols(Wt):  # [K, 512] torch-order cols; g-gate 2x fold
        blocks = [Wt[:, a:b] for (a, b) in _GATE_ROWS]
        blocks[2] = 2.0 * blocks[2]  # tanh(g) from exp(-2g)
        return _f32(np.concatenate(blocks, 1))

    def wih_feature_rows(Wih):  # [512, 108] -> feature-major [108, 512]
        Wt = _f32(Wih).T
        out = np.zeros((108, 4 * H), np.float32)
        for r in range(RSS):
            out[96 + r] = Wt[r * 9]
            for j in range(FEAT):
                out[r * 8 + j] = Wt[r * 9 + 1 + j]
        return out

    bsum0 = _f32(inputs["bih0"]) + _f32(inputs["bhh0"])
    bsum1 = _f32(inputs["bih1"]) + _f32(inputs["bhh1"])
    Wih0T = np.concatenate([wih_feature_rows(inputs["Wih0"]),
                            bsum0[None, :]], 0)  # [109, 512]

    # layer-1 bias via exp(-(v+b)) = exp(-v)*exp(-b); g block pre-doubled
    eb1 = np.zeros((H, 4), np.float32)
    for gi, (a, b_) in enumerate(_GATE_ROWS):
        s = -1.0 if gi < 3 else -2.0
        eb1[:, gi] = np.exp(s * bsum1[a:b_])

    # consts: name -> (array, dtype). bf16 for every matmul operand
    # except the fp32 dist path (poshT) + DVE-side scalars.
    consts = {
        "qW1a": (_lowp(_f32(inputs["q_W1"]).T[0:1]), LOWP),       # [1, 64]
        "qW1b": (_lowp(_f32(inputs["q_W1"]).T[1:4]), LOWP),       # [3, 64]
        "b1q": (_f32(inputs["q_b1"])[:, None], F32),              # [64, 1]
        "b1f": ((_f32(inputs["fc_W1"]) @ _f32(inputs["ln_b"])
                 + _f32(inputs["fc_b1"]))[:, None], F32),          # [64, 1]
        "KWT_e": (_lowp(KWT_e), LOWP),                            # [65, LP]
        "poshT": (_lowp(poshT), LOWP),                            # [35, LP]
        "Ebc": (_lowp(Ebc), LOWP),                                # [12, 96]
        "Wih0T": (_lowp(gate_perm_cols(Wih0T)), LOWP),            # [109, 512]
        "Whh0T": (_lowp(gate_perm_cols(_f32(inputs["Whh0"]).T)), LOWP),
        "Wih1T": (_lowp(gate_perm_cols(_f32(inputs["Wih1"]).T)), LOWP),
        "Whh1T": (_lowp(gate_perm_cols(_f32(inputs["Whh1"]).T)), LOWP),
        "W1T": (_lowp((_f32(inputs["fc_W1"])
                       * _f32(inputs["ln_g"])[None, :]).T), LOWP),  # [128, 64]
        "W2T": (_lowp(np.concatenate([_f32(inputs["fc_W2"]).T,
                                      _f32(inputs["fc_b2"])[None, :]],
                                     0)), LOWP),                   # [65, 3]
        "ones128": (_lowp(np.ones((128, 1))), LOWP),
        "ones13": (_lowp(np.ones((1, 3))), LOWP),
        "epsd": (np.full((128, 1), DIST_EPS, np.float32), F32),
        "epsl": (np.full((1, 1), LN_EPS, np.float32), F32),
    }
    for c in range(NC):
        consts[f"XW{c}"] = (_lowp(XW[c]), LOWP)

    consts["w1s"] = (_lowp(_f32((_f32(inputs["fc_W1"])
                                 * _f32(inputs["ln_g"])[None, :]).T
                                .sum(0, keepdims=True))), LOWP)    # [1, 64]

    pred0 = gpf_p[:, :3].mean(0).astype(np.float32)
    pred0c = pred0 - CEN
    init = {
        "prevaux0": (_lowp(np.concatenate(
            [np.broadcast_to(pred0c[:, None], (3, BL)),
             np.ones((1, BL), np.float32),
             np.zeros((28, BL), np.float32),
             np.broadcast_to((pred0c * pred0c)[:, None], (3, BL))],
            0)), LOWP),                                            # [35, BL]
        "prev30": (_lowp(np.broadcast_to(pred0[:, None], (3, BL))), LOWP),
        "h00": (_lowp(np.zeros((H, BL))), LOWP),
        "h10": (_lowp(np.zeros((H, BL))), LOWP),
        "c00": (np.zeros((H, BL), np.float32), F32),
        "c10": (np.zeros((H, BL), np.float32), F32),
        "q1re0": (_lowp(np.concatenate(
            [np.zeros((64, RSS * BL)), np.ones((1, RSS * BL))], 0)), LOWP),
        "xT0": (_lowp(np.r_[np.zeros((108, BL)), np.ones((1, BL))]), LOWP),
        "Are0": (_lowp(np.r_[np.zeros((64, BL)), np.ones((1, BL))]), LOWP),
    }
    # Pad the slot list so every 32-row band of every chunk is written:
    # unwritten PSUM rows would feed exp() with garbage (NaN via inf*0 in
    # the agg matmul). Dummy slots read KWT_e's zero columns -> scores 0,
    # u = exp(-0.5*ln(dummy d^2)) ~ 1e-6, and XW rows there are zero.
    used = {gb for (gb, _) in slot_of_group}
    slots_full = list(slot_of_group) + [(gb, 0) for gb in range(0, LP, 32)
                                        if gb not in used]
    meta = {
        "slots": slots_full,      # [(padded_base, group_r)] covers all LP
        "NC": NC,
        "LP": LP,
        # zero-bias fast paths: relu as a DVE max (no bias add needed)
        "b1q_zero": bool(not np.any(_f32(inputs["q_b1"])))
                    and not os.environ.get("KRELUACT"),
        "b1f_zero": bool(not np.any(_f32(inputs["fc_W1"]) @ _f32(inputs["ln_b"])
                                    + _f32(inputs["fc_b1"])))
                    and not os.environ.get("KRELUACT"),
    }
    return consts, init, meta


def _per_core_rss(rss_core):
    """rss_core [BL, T, RSS] -> rss_q [T, RSS*BL] and rssT [12, T*BL]."""
    rss_q = np.ascontiguousarray(
        rss_core.transpose(1, 2, 0).reshape(T, RSS * BL)).astype(NPLP)
    rssT = np.ascontiguousarray(
        rss_core.transpose(2, 1, 0).reshape(RSS, T * BL)).astype(NPLP)
    return rss_q, rssT


def build_nc(consts, init, meta, nsteps=T):
    """Build the per-core Bass program (SPMD across the 8 cores)."""
    nc = bacc.Bacc("TRN2", target_bir_lowering=False, debug=False,
                   num_devices=NCORES)
    R = RSS * BL  # 384
    NC = meta["NC"]
    slots = meta["slots"]

    dram = {}
    for k, (v, dt_) in {**consts, **init}.items():
        dram[k] = nc.dram_tensor(k, list(v.shape), dt_,
                                 kind="ExternalInput").ap()
    dram["rss_q"] = nc.dram_tensor("rss_q", [T, R], LOWP,
                                   kind="ExternalInput").ap()
    dram["rssT"] = nc.dram_tensor("rssT", [RSS, T * BL], LOWP,
                                  kind="ExternalInput").ap()
    d_out = nc.dram_tensor("out", [3, nsteps * BL], F32,
                           kind="ExternalOutput").ap()

    with tile.TileContext(nc) as tc:
        with (
            tc.tile_pool(name="const", bufs=1) as cpool,
            tc.tile_pool(name="state", bufs=1) as spool,
            tc.tile_pool(name="work", bufs=2) as wpool,
            tc.tile_pool(name="lnp", bufs=2) as lnpool,
            tc.tile_pool(name="qrow", bufs=3) as qpool,
            # PSUM: 8 banks total. q1(1) + scores(1) + dist/r3(1) + P(1)
            # + gates(3, cross-step prefetch) + small(1) = 8.
            tc.tile_pool(name="pq1", bufs=2, space="PSUM") as pq1,
            tc.tile_pool(name="psc", bufs=1, space="PSUM") as psc,
            tc.tile_pool(name="pds", bufs=1, space="PSUM") as pds,
            tc.tile_pool(name="pP", bufs=1, space="PSUM") as pP,
            # pg=2 suffices: whh(t+1) reuses a gate bank only after its
            # e0/eo readers completed (mid-step). The freed bank double-
            # buffers q1 so the hoisted q1a matmul never queues between
            # the score matmuls and the diff subtract's sem threshold.
            tc.tile_pool(name="pg", bufs=2, space="PSUM") as pg,
            tc.tile_pool(name="pst", bufs=1, space="PSUM") as pst,
            tc.tile_pool(name="pbc", bufs=1, space="PSUM") as pbc,
        ):
            cs = {}
            for k, (v, dt_) in consts.items():
                t_ = cpool.tile(list(v.shape), dt_, tag=k, name=k)
                nc.sync.dma_start(t_[:], dram[k][:])
                cs[k] = t_
            t_rssT = cpool.tile([RSS, T * BL], LOWP, tag="rssT", name="t_rssT")
            nc.sync.dma_start(t_rssT[:], dram["rssT"][:])

            st = {}
            for k, shape, dt_ in [("prevaux", [35, BL], LOWP),
                                  ("prev3", [3, BL], LOWP),
                                  ("h0", [H, BL], LOWP),
                                  ("h1w", [H, 2 * BL], LOWP),
                                  ("c0", [H, BL], F32), ("c1", [H, BL], F32),
                                  ("q1re", [65, R], LOWP),
                                  ("xT", [109, BL], LOWP),
                                  ("Are", [65, BL], LOWP)]:
                st[k] = spool.tile(shape, dt_, tag=k, name="st_" + k)
            for k, src in [("prevaux", "prevaux0"), ("prev3", "prev30"),
                           ("h0", "h00"),
                           ("c0", "c00"), ("c1", "c10"),
                           ("q1re", "q1re0"), ("xT", "xT0"), ("Are", "Are0")]:
                nc.sync.dma_start(st[k][:], dram[src][:])
            nc.sync.dma_start(st["h1w"][:, 0:BL], dram["h10"][:])
            nc.sync.dma_start(st["h1w"][:, BL:2 * BL], dram["h10"][:])
            t_out = spool.tile([3, nsteps * BL], F32, tag="out_sb",
                               name="t_out")

            mm = nc.tensor.matmul
            act = nc.scalar.activation
            V = nc.vector

            def bc_r(ap3):
                """[3, BL] AP -> broadcast [3, RSS*BL] (free [[0,12],[1,BL]])."""
                return bass.AP(ap3.tensor, ap3.offset,
                               [ap3.ap[0], [0, RSS], ap3.ap[-1]])

            def gate_mms(gps, W, x, start, stop):
                # start=True ONLY on the first mm: the 0x1 flag clears
                # has_written BANK-WIDE, so a second start would flip the
                # already-written gate regions back to overwrite mode and
                # the later wih accumulation would silently drop them.
                for gi in range(4):
                    mm(gps[:, gi * BL:(gi + 1) * BL],
                       W[:, gi * H:(gi + 1) * H], x[:],
                       start=(start and gi == 0), stop=stop,
                       skip_group_check=True)

            PREFETCH = not os.environ.get("KNOPREFETCH")

            # ---- step-0 prefetches (the loop does these for t+1) ----
            qr_next = qpool.tile([1, R], LOWP, tag="qrow", name="qrow")
            nc.sync.dma_start(qr_next[:], dram["rss_q"][0:1, :])
            if PREFETCH:
                g0_next = pg.tile([128, 4 * BL], F32, tag="g", name="gps")
                gate_mms(g0_next, cs["Whh0T"], st["h0"], True, False)
                g1_next = pg.tile([128, 4 * BL], F32, tag="g", name="gps")
                gate_mms(g1_next, cs["Whh1T"], st["h1w"][:, 0:BL], True, False)

            for t in range(nsteps):
                prev = st["prevaux"][0:3, :] if t == 0 else \
                    t_out[:, (t - 1) * BL:t * BL]
                qrow = qr_next
                if PREFETCH:
                    gps0 = g0_next
                    gps1 = g1_next

                # xT rss rows: depends only on t; off the critical path on
                # the (otherwise idle) GpSimd engine
                GP = V if os.environ.get("KNOGPS") else nc.gpsimd
                GP.tensor_copy(st["xT"][96:108, :],
                               t_rssT[:, t * BL:(t + 1) * BL])

                # ---------- q-MLP + grouped scores ----------
                # q1a (rss half) emitted adjacent to q1b so the PSUM
                # accumulation group is textually contiguous (a cross-step
                # split group got its start flag mangled by the compiler);
                # the dep scheduler still hoists q1a early (needs only the
                # qrow DMA + the bank's WAR).
                q1ps = pq1.tile([64, R], F32, tag="q1", name="q1ps")
                mm(q1ps[:], cs["qW1a"][:], qrow[:],
                   start=True, stop=False, skip_group_check=True)
                mm(q1ps[:], cs["qW1b"][:], bc_r(st["prev3"][:]),
                   start=False, stop=True, skip_group_check=True)
                if meta["b1q_zero"]:
                    if os.environ.get("KRSPLIT"):
                        V.tensor_scalar_max(st["q1re"][0:64, 0:R // 2],
                                            q1ps[:, 0:R // 2], 0.0)
                        act(st["q1re"][0:64, R // 2:R], q1ps[:, R // 2:R],
                            AFT.Relu, bias=0.0)
                    else:
                        V.tensor_scalar_max(st["q1re"][0:64, :],
                                            q1ps[:], 0.0)
                else:
                    act(st["q1re"][0:64, :], q1ps[:], AFT.Relu,
                        bias=cs["b1q"][:, 0:1])

                # start=True ONLY on the first slot mm (bank-wide has_written
                # clear); the rest write fresh disjoint regions with 0x0
                # (has_written=false -> overwrite) and SET has_written, so
                # the -lnd matmul below accumulates everywhere.
                # NOTE: these column-tiled (tile_position) mms must keep
                # per-slot start=True/stop=True — a first-only-start scheme
                # (to let a later matmul accumulate -lnd into this bank)
                # corrupts the PSUM on HW.
                spsum = psc.tile([128, NC * BL], F32, tag="sc", name="spsum")
                for (gb, r) in slots:
                    c, lb = divmod(gb, 128)
                    mm(spsum[lb:lb + 32, c * BL:(c + 1) * BL],
                       cs["KWT_e"][:, gb:gb + 32],
                       st["q1re"][:, r * BL:(r + 1) * BL],
                       start=True, stop=True, tile_position=(0, lb))

                # ---------- distance term (fp32: cancellation) ----------
                dps = pds.tile([128, NC * BL], F32, tag="ds", name="dps")
                for c in range(NC):
                    sl = slice(c * 128, (c + 1) * 128)
                    mm(dps[:, c * BL:(c + 1) * BL], cs["poshT"][:, sl],
                       st["prevaux"][:, :], start=True, stop=True)

                if t + 1 < nsteps:
                    qr_next = qpool.tile([1, R], LOWP, tag="qrow",
                                         name="qrow")
                    nc.sync.dma_start(qr_next[:],
                                      dram["rss_q"][t + 1:t + 2, :])

                u_sb = wpool.tile([128, NC * BL], LOWP, tag="u", name="u_sb")
                lnd = wpool.tile([128, NC * BL], F32, tag="lnd", name="lnd")
                act(lnd[:], dps[:], AFT.Ln, bias=cs["epsd"][:, 0:1])
                diff = wpool.tile([128, NC * BL], F32, tag="diff",
                                  name="diff")
                V.tensor_sub(diff[:], spsum[:], lnd[:])
                act(u_sb[:], diff[:], AFT.Exp, scale=0.5)

                # ---------- aggregate + normalize ----------
                Pps = pP.tile([108, BL], F32, tag="P", name="Pps")
                for c in range(NC):
                    mm(Pps[:], cs[f"XW{c}"][:],
                       u_sb[:, c * BL:(c + 1) * BL],
                       start=(c == 0), stop=(c == NC - 1))

                # NOTE: reciprocal_approx_fast mis-reads PSUM operands
                # (BITWISE_NOT custom-DVE path); keep the exact iterative
                # reciprocal here — [12, 32] is cheap anyway. fp16 output
                # so the Ebc broadcast matmul runs 1-pass.
                rT = wpool.tile([RSS, BL], LOWP, tag="rT", name="rT")
                with nc.allow_low_precision(reason="softmax 1/sum -> fp16"):
                    V.reciprocal(rT[:], Pps[96:108, :])
                sbps = pst.tile([96, BL], F32, tag="small", name="sbps")
                mm(sbps[:], cs["Ebc"][:], rT[:], start=True, stop=True)
                sb96 = wpool.tile([96, BL], F32, tag="sb96", name="sb96")
                V.tensor_copy(sb96[:], sbps[:])
                V.tensor_mul(st["xT"][0:96, :], Pps[0:96, :], sb96[:])
                dbg_tiles = {"lnd": lnd, "diff": diff, "u": u_sb,
                             "sb96": sb96, "rT": rT}
                if os.environ.get("KXTLATE"):
                    V.tensor_copy(st["xT"][96:108, :],
                                  t_rssT[:, t * BL:(t + 1) * BL])

                # ---------- two LSTM layers ----------
                for ly in range(2):
                    wih = cs["Wih0T"] if ly == 0 else cs["Wih1T"]
                    xin = st["xT"] if ly == 0 else st["h0"]
                    cst = st["c0"] if ly == 0 else st["c1"]
                    if PREFETCH:
                        gps = gps0 if ly == 0 else gps1
                        gate_mms(gps, wih, xin, False, True)
                    else:
                        whh = cs["Whh0T"] if ly == 0 else cs["Whh1T"]
                        hin = st["h0"] if ly == 0 else st["h1w"][:, 0:BL]
                        gps = pg.tile([128, 4 * BL], F32, tag="g",
                                      name="gps")
                        gate_mms(gps, whh, hin, True, False)
                        gate_mms(gps, wih, xin, False, True)

                    # i,f,g sigmoids feed the cell update; o's sigmoid is
                    # computed while exp(-2c) sits on the ACT engine.
                    e0 = wpool.tile([128, 3 * BL], F32, tag=f"e{ly}",
                                    name="e0t")
                    act(e0[:], gps[:, 0:3 * BL], AFT.Exp, scale=-1.0)
                    eo = wpool.tile([128, BL], F32, tag=f"eo{ly}", name="eot")
                    act(eo[:], gps[:, 3 * BL:4 * BL], AFT.Exp, scale=-1.0)
                    ea = wpool.tile([128, 3 * BL], F32, tag=f"ea{ly}",
                                    name="eat")
                    V.tensor_scalar_add(ea[:], e0[:], 1.0)
                    rg = wpool.tile([128, 3 * BL], F32, tag=f"rg{ly}",
                                    name="rgt")
                    V.reciprocal_approx_fast(rg[:], ea[:])
                    tg = wpool.tile([128, BL], F32, tag=f"tg{ly}", name="tgt")
                    V.tensor_scalar(tg[:], rg[:, 2 * BL:3 * BL], 2.0, 1.0,
                                    op0=ALU.mult, op1=ALU.subtract)
                    p_ = wpool.tile([128, BL], F32, tag=f"p{ly}", name="p_t")
                    V.tensor_mul(p_[:], rg[:, BL:2 * BL], cst[:])  # sig(f)*c
                    q_ = wpool.tile([128, BL], F32, tag=f"q{ly}", name="q_t")
                    V.tensor_mul(q_[:], rg[:, 0:BL], tg[:])        # sig(i)*tg
                    V.tensor_add(cst[:], p_[:], q_[:])

                    ec = wpool.tile([128, BL], F32, tag=f"ec{ly}", name="ect")
                    act(ec[:], cst[:], AFT.Exp, scale=-2.0)
                    # o-gate sigmoid on DVE during the exp(-2c) latency
                    eao = wpool.tile([128, BL], F32, tag=f"eao{ly}",
                                     name="eaot")
                    V.tensor_scalar_add(eao[:], eo[:], 1.0)
                    ro = wpool.tile([128, BL], F32, tag=f"ro{ly}", name="rot")
                    V.reciprocal_approx_fast(ro[:], eao[:])
                    eac = wpool.tile([128, BL], F32, tag=f"eac{ly}",
                                     name="eact")
                    V.tensor_scalar_add(eac[:], ec[:], 1.0)
                    rc = wpool.tile([128, BL], F32, tag=f"rc{ly}", name="rct")
                    V.reciprocal_approx_fast(rc[:], eac[:])
                    thc = wpool.tile([128, BL], F32, tag=f"thc{ly}",
                                     name="thct")
                    V.tensor_scalar(thc[:], rc[:], 2.0, 1.0,
                                    op0=ALU.mult, op1=ALU.subtract)
                    hout = st["h0"][:] if ly == 0 else st["h1w"][:, 0:BL]
                    V.tensor_mul(hout, ro[:], thc[:])

                    # prefetch next step's Whh gates the moment h is out
                    if PREFETCH and ly == 0 and t + 1 < nsteps:
                        g0_next = pg.tile([128, 4 * BL], F32, tag="g",
                                          name="gps")
                        gate_mms(g0_next, cs["Whh0T"], st["h0"], True, False)

                # ---------- LayerNorm + fc head ----------
                # stat split: the sum(h) half issues right at h1 so the
                # -mu/m2/vv/rstd pole starts ~350ns earlier; sum(h^2)
                # follows the h^2 multiply. Emitted BEFORE the whh1
                # prefetch so the in-order PE doesn't queue 4 gate
                # matmuls ahead of them.
                stat = pst.tile([1, 2 * BL], F32, tag="small", name="stat")
                mm(stat[0:1, 0:BL], cs["ones128"][:], st["h1w"][:, 0:BL],
                   start=True, stop=True)
                V.tensor_mul(st["h1w"][:, BL:2 * BL], st["h1w"][:, 0:BL],
                             st["h1w"][:, 0:BL])
                mm(stat[0:1, BL:2 * BL], cs["ones128"][:],
                   st["h1w"][:, BL:2 * BL], start=True, stop=True)
                if PREFETCH and t + 1 < nsteps:
                    g1_next = pg.tile([128, 4 * BL], F32, tag="g",
                                      name="gps")
                    gate_mms(g1_next, cs["Whh1T"], st["h1w"][:, 0:BL],
                             True, False)

                bsrc = lnpool.tile([1, 2 * BL], LOWP, tag="bsrc", name="bsrc")
                V.tensor_scalar_mul(bsrc[0:1, 0:BL], stat[0:1, 0:BL],
                                    -1.0 / H)  # -mu (fp16 is plenty)
                m2 = lnpool.tile([1, BL], F32, tag="m2", name="m2")
                V.tensor_mul(m2[:], bsrc[0:1, 0:BL], bsrc[0:1, 0:BL])
                vv = lnpool.tile([1, BL], F32, tag="vv", name="vv")
                V.scalar_tensor_tensor(vv[:], stat[0:1, BL:2 * BL], 1.0 / H,
                                       m2[:], op0=ALU.mult, op1=ALU.subtract)
                lv = lnpool.tile([1, BL], F32, tag="lv", name="lv")
                act(lv[:], vv[:], AFT.Ln, bias=cs["epsl"][:, 0:1])
                act(bsrc[0:1, BL:2 * BL], lv[:], AFT.Exp, scale=-0.5)

                # relu(rstd*(W1g@(h1-mu))) = rstd*relu(W1g@h1 - mu*w1s):
                # centering is a K=1 accumulating matmul; rstd scales the
                # (bias-free) head output at the very end.
                a2ps = pst.tile([64, BL], F32, tag="small", name="a2ps")
                mm(a2ps[:], cs["W1T"][:], st["h1w"][:, 0:BL],
                   start=True, stop=False)
                mm(a2ps[:], cs["w1s"][:], bsrc[0:1, 0:BL],
                   start=False, stop=True)
                if meta["b1f_zero"]:
                    V.tensor_scalar_max(st["Are"][0:64, :], a2ps[:], 0.0)
                else:
                    act(st["Are"][0:64, :], a2ps[:], AFT.Relu,
                        bias=cs["b1f"][:, 0:1])
                prps = pst.tile([3, BL], F32, tag="small", name="prps")
                mm(prps[:], cs["W2T"][:], st["Are"][:], start=True, stop=True)
                # r3 broadcast reuses the dist PSUM bank (free this late
                # in the step; next dist write waits for r3sb's read)
                if os.environ.get("KR3SEP"):
                    r3ps = pbc.tile([3, BL], F32, tag="bc", name="r3ps")[:]
                else:
                    r3ps = dps[0:3, 0:BL]
                mm(r3ps, cs["ones13"][:], bsrc[0:1, BL:2 * BL],
                   start=True, stop=True)
                r3sb = lnpool.tile([3, BL], F32, tag="r3sb", name="r3sb")
                V.tensor_copy(r3sb[:], r3ps)
                nxt = t_out[:, t * BL:(t + 1) * BL]
                if t + 1 < nsteps:
                    V.tensor_mul(nxt, prps[:], r3sb[:])
                    if os.environ.get("KPREV3CAST"):
                        V.tensor_copy(st["prev3"][:], nxt)
                    else:
                        V.tensor_mul(st["prev3"][:], prps[:], r3sb[:])
                    V.tensor_scalar_add(st["prevaux"][0:3, :], nxt, -CEN)
                    V.tensor_mul(st["prevaux"][32:35, :],
                                 st["prevaux"][0:3, :],
                                 st["prevaux"][0:3, :])
                else:
                    V.tensor_mul(nxt, prps[:], r3sb[:])
                    q1_next = pq1.tile([64, R], F32, tag="q1", name="q1ps")
                    mm(q1_next[:], cs["qW1a"][:], qr_next[:],
                       start=True, stop=False, skip_group_check=True)

            if os.environ.get("KDBG"):
                for nm, ap, shp, dt_ in [
                        ("d_lnd", dbg_tiles["lnd"], [128, NC * BL], F32),
                        ("d_diff", dbg_tiles["diff"], [128, NC * BL], F32),
                        ("d_u", dbg_tiles["u"], [128, NC * BL], LOWP),
                        ("d_sb96", dbg_tiles["sb96"], [96, BL], F32),
                        ("d_rT", dbg_tiles["rT"], [RSS, BL], LOWP),
                        ("d_prevaux", st["prevaux"], [35, BL], LOWP),
                        ("d_q1re", st["q1re"], [65, R], LOWP),
                        ("d_xT", st["xT"], [109, BL], LOWP),
                        ("d_h0", st["h0"], [H, BL], LOWP),
                        ("d_c0", st["c0"], [H, BL], F32),
                        ("d_h1w", st["h1w"], [H, 2 * BL], LOWP),
                        ("d_Are", st["Are"], [65, BL], LOWP),
                        ("d_prev3", st["prev3"], [3, BL], LOWP)]:
                    dd = nc.dram_tensor(nm, shp, dt_,
                                        kind="ExternalOutput").ap()
                    nc.sync.dma_start(dd[:], ap[:])
            nc.sync.dma_start(d_out[:], t_out[:])

    nc.compile()
    return nc


def make_in_maps(consts, init, rss_seq):
    base = {k: v for k, (v, _) in {**consts, **init}.items()}
    in_maps = []
    for k in range(NCORES):
        rss_q, rssT = _per_core_rss(rss_seq[k * BL:(k + 1) * BL])
        m = dict(base)
        m["rss_q"] = rss_q
        m["rssT"] = rssT
        in_maps.append(m)
    return in_maps


def kernel(**inputs):
    rss_seq = _f32(inputs["rss_seq"])
    consts, init, meta = _host_prep(inputs)
    nc = build_nc(consts, init, meta, nsteps=T)
    in_maps = make_in_maps(consts, init, rss_seq)
    res = run_bass_kernel_spmd(nc, in_maps, core_ids=list(range(NCORES)))
    outs = []
    for k in range(NCORES):
        o = res.results[k]["out"]
        outs.append(np.asarray(o).reshape(3, T, BL).transpose(2, 1, 0))
    return np.ascontiguousarray(np.concatenate(outs, 0))


# revision 68
# speedup vs baseline: 1.0055x; 1.0055x over previous
"""Trainium2 Bass kernel for nn_Attentive_VLP_LSTM.

kernel(**inputs) takes the FULL unsharded inputs (numpy) and returns the
FULL [B, T, 3] output. Batch is sharded over 8 NeuronCores (32 rows
each); each core runs a fully-unrolled T=256 recurrent Bass/Tile program.

Design notes (v3 — fp16 matmuls + software pipelining):
  - All matmuls run in fp16 (1 PE pass @ 1 cyc/row vs fp32's 2 passes @
    4 cyc/row; fp16 over bf16 for the 8x finer mantissa at equal speed).
    The distance matmul survives fp16 by recentering all positions by
    CEN=2.5 (shrinks the |led|^2-scale terms whose cancellation dominates
    the quantization error of d^2).
  - Single ACT table set (natural_log_exp_and_others): softmax uses
    exp(q.k - 0.5*ln(d^2+eps)); LayerNorm rstd = exp(-0.5*ln(var+eps));
    LSTM sigmoid/tanh built from exp + DVE reciprocal_approx_fast.
    Gate layout is (i,f,g,o): the o-gate sigmoid runs on DVE during the
    exp(-2c) ACT latency. Relu runs as a DVE max when biases are zero.
  - Cross-step software pipelining: the Whh gate matmuls for step t+1
    are issued as soon as h(t) is ready (they hide inside step t's
    DVE/ACT phase). The framework list-schedules by dependencies, so the
    rss half of the q-MLP hoists itself off the critical path.
  - PSUM accumulation-group pitfalls (hardware-verified):
    * start=True clears has_written beyond the instruction's own write
      region - a multi-region group must put start=True ONLY on its
      first matmul, or later accumulating writes silently overwrite.
    * a group whose start/stop halves are textually far apart (emitted
      across the step boundary) can lose its start flag entirely in
      compilation; keep groups contiguous in emission order.
    * DVE/custom-DVE ops must not read two PSUM operands, and
      reciprocal_approx_fast mis-reads PSUM inputs outright.
  - LEDs are sorted by frequency and laid out in 32-aligned slots per
    frequency group (matmul tile_position needs 32-aligned bases); the
    per-(batch, rss-row) softmax runs only over its own group; all 16
    slots are emitted (dummies hit KWT_e's zero columns) so no PSUM row
    feeds exp() uninitialized. Group sums + led_feat aggregation are one
    block-diagonal matmul (XW).
  - Activations are feature-major [feat, batch]; biases are folded into
    matmuls via ones-rows. Output accumulates in SBUF [3, T*32], host
    transposes. r3 broadcast PSUM shares the dist bank (disjoint in
    time) so everything fits in 8 PSUM banks.
"""

import os
import sys

import numpy as np

for _p in ("/opt/trn_rl_repo", "/root/.axon_site", "/root/.axon_site/_ro/pypackages"):
    if _p not in sys.path and os.path.isdir(_p):
        sys.path.append(_p)

import ml_dtypes

import concourse.bass as bass
import concourse.tile as tile
from concourse import bacc, mybir
from concourse.bass_utils import run_bass_kernel_spmd

# The act-table chooser assigns each activation the FIRST table set that
# contains its function, which thrashes between `natural_log` and
# `exp_and_others` (one ~1.3us reload per switch, several per step).
# Every ACT function this kernel uses (Exp, Ln, Relu) lives in
# `natural_log_exp_and_others`, so blank out every other set (order and
# indices preserved -> act_func_set_id stays correct) to get ONE load.
_KEEP_ACT_SET = "natural_log_exp_and_others"
_orig_get_act_tables = bacc.get_activation_tables


def _patched_get_act_tables(arch):
    tabs = _orig_get_act_tables(arch)
    return {name: (fns if name == _KEEP_ACT_SET else set())
            for name, fns in tabs.items()}


bacc.get_activation_tables = _patched_get_act_tables

B, T, RSS, L, FEAT, H = 256, 256, 12, 256, 8, 128
NCORES = 8
BL = B // NCORES  # 32 batch rows per core
AFT = mybir.ActivationFunctionType
ALU = mybir.AluOpType
F32 = mybir.dt.float32
LOWP = mybir.dt.float16   # fp16: 1 cyc/row on PE like bf16, 8x finer mantissa
NPLP = np.float16
if os.environ.get("KF32"):        # debug: full fp32 matmuls
    LOWP = mybir.dt.float32
    NPLP = np.float32
DIST_EPS = 1e-8
LN_EPS = 1e-5
CEN = 2.5  # room-center shift for the fp16 distance matmul

# Gate column-block order in the [128, 4*BL] gate psum: i, f, g, o
# (torch weight-row order, kept as-is). g pre-doubled for the tanh fold;
# o last so its sigmoid chain can overlap the exp(-2c) ACT latency.
_GATE_ROWS = [(0, H), (H, 2 * H), (2 * H, 3 * H), (3 * H, 4 * H)]


def _f32(x):
    return np.ascontiguousarray(np.asarray(x, np.float32))


def _lowp(x):
    return np.ascontiguousarray(np.asarray(x, NPLP))


def _host_prep(inputs):
    """All static marshalling: LED sort + 32-aligned slot layout, small
    one-time MLPs, weight folds."""
    gpf = _f32(inputs["global_led_pos_freq"])  # [L, 4]
    freq = gpf[:, 3]
    perm = np.argsort(freq, kind="stable")
    gpf_p = gpf[perm]

    relu = lambda x: np.maximum(x, np.float32(0))
    lin = lambda x, W, bb: x @ _f32(W).T + _f32(bb)

    led_feat = relu(lin(relu(lin(gpf_p, inputs["enc_W1"], inputs["enc_b1"])),
                        inputs["enc_W2"], inputs["enc_b2"]))  # [L, 8]
    keys = lin(relu(lin(np.concatenate([led_feat, gpf_p[:, :3]], 1),
                        inputs["k_W1"], inputs["k_b1"])),
               inputs["k_W2"], inputs["k_b2"])  # [L, 64]

    # --- padded slot layout: each freq group starts at a 32-boundary ---
    freq_p = gpf_p[:, 3]
    bounds = np.searchsorted(freq_p, np.arange(1, RSS + 2) - 0.5)
    slot_of_group = []   # list of (slot_base, r) 32-wide scores-MM slots
    src_list = []
    base = 0
    for r in range(RSS):
        g0, g1 = int(bounds[r]), int(bounds[r + 1])
        n = g1 - g0
        nslots = max(1, (n + 31) // 32)
        for s in range(nslots):
            slot_of_group.append((base + 32 * s, r))
        sl = -np.ones(nslots * 32, np.int64)
        sl[:n] = np.arange(g0, g1)
        src_list.append(sl)
        base += nslots * 32
    pad_src = np.concatenate(src_list)  # padded-pos -> sorted-led idx or -1
    LP = ((base + 127) // 128) * 128    # padded led count, 128-multiple
    pad_src = np.concatenate([pad_src, -np.ones(LP - base, np.int64)])
    NC = LP // 128
    real = pad_src >= 0

    def expand(arr_p, fill=0.0):
        """[L, ...] sorted-led array -> [LP, ...] padded."""
        out = np.full((LP,) + arr_p.shape[1:], fill, np.float32)
        out[real] = arr_p[pad_src[real]]
        return out

    led_feat_e = expand(led_feat)
    keys_e = expand(keys)
    pos_e = expand(gpf_p[:, :3], fill=100.0)  # dummies far away (fp16-safe)
    r_of_e = np.full(LP, 0, np.int64)
    r_of_e[real] = np.rint(freq_p[pad_src[real]] - 1.0).astype(np.int64)

    KW = 2.0 * keys_e @ _f32(inputs["q_W2"])      # [LP, 64]
    kb2 = 2.0 * keys_e @ _f32(inputs["q_b2"])     # [LP]
    KWT_e = _f32(np.concatenate([KW.T, kb2[None, :]], 0))  # [65, LP]

    # prevaux rows: 0:3 pred-CEN, 3 ones, 4:32 zero pad, 32:35 (pred-CEN)^2
    # (compute-engine writes must start at 32-aligned partitions).
    # Positions are recentered by CEN so the d^2 matmul is fp16-safe:
    # |led|^2-scale terms shrink ~4x, keeping the catastrophic-cancellation
    # error of quantized inputs well below ln()'s sensitivity.
    pos_c = pos_e - CEN
    poshT = np.zeros((35, LP), np.float32)
    poshT[0:3] = -2.0 * pos_c.T
    poshT[3] = (pos_c * pos_c).sum(1)
    poshT[32:35] = 1.0

    # XW [NC][128, 108]: cols 0:96 = agg (r*8+j), 96:108 = group sums
    XW = np.zeros((NC, 128, 108), np.float32)
    for lp in range(LP):
        if not real[lp]:
            continue
        c, lr = divmod(lp, 128)
        r = r_of_e[lp]
        XW[c, lr, r * 8:(r + 1) * 8] = led_feat_e[lp]
        XW[c, lr, 96 + r] = 1.0

    Ebc = np.zeros((RSS, 96), np.float32)
    for r in range(RSS):
        Ebc[r, r * 8:(r + 1) * 8] = 1.0

    def gate_perm_cols(Wt):  # [K, 512] torch-order cols; g-gate 2x fold
        blocks = [Wt[:, a:b] for (a, b) in _GATE_ROWS]
        blocks[2] = 2.0 * blocks[2]  # tanh(g) from exp(-2g)
        return _f32(np.concatenate(blocks, 1))

    def wih_feature_rows(Wih):  # [512, 108] -> feature-major [108, 512]
        Wt = _f32(Wih).T
        out = np.zeros((108, 4 * H), np.float32)
        for r in range(RSS):
            out[96 + r] = Wt[r * 9]
            for j in range(FEAT):
                out[r * 8 + j] = Wt[r * 9 + 1 + j]
        return out

    bsum0 = _f32(inputs["bih0"]) + _f32(inputs["bhh0"])
    bsum1 = _f32(inputs["bih1"]) + _f32(inputs["bhh1"])
    Wih0T = np.concatenate([wih_feature_rows(inputs["Wih0"]),
                            bsum0[None, :]], 0)  # [109, 512]

    # layer-1 bias via exp(-(v+b)) = exp(-v)*exp(-b); g block pre-doubled
    eb1 = np.zeros((H, 4), np.float32)
    for gi, (a, b_) in enumerate(_GATE_ROWS):
        s = -1.0 if gi < 3 else -2.0
        eb1[:, gi] = np.exp(s * bsum1[a:b_])

    # consts: name -> (array, dtype). bf16 for every matmul operand
    # except the fp32 dist path (poshT) + DVE-side scalars.
    consts = {
        "qW1a": (_lowp(_f32(inputs["q_W1"]).T[0:1]), LOWP),       # [1, 64]
        "qW1b": (_lowp(_f32(inputs["q_W1"]).T[1:4]), LOWP),       # [3, 64]
        "b1q": (_f32(inputs["q_b1"])[:, None], F32),              # [64, 1]
        "b1f": ((_f32(inputs["fc_W1"]) @ _f32(inputs["ln_b"])
                 + _f32(inputs["fc_b1"]))[:, None], F32),          # [64, 1]
        "KWT_e": (_lowp(KWT_e), LOWP),                            # [65, LP]
        "poshT": (_lowp(poshT), LOWP),                            # [35, LP]
        "Ebc": (_lowp(Ebc), LOWP),                                # [12, 96]
        "Wih0T": (_lowp(gate_perm_cols(Wih0T)), LOWP),            # [109, 512]
        "Whh0T": (_lowp(gate_perm_cols(_f32(inputs["Whh0"]).T)), LOWP),
        "Wih1T": (_lowp(gate_perm_cols(_f32(inputs["Wih1"]).T)), LOWP),
        "Whh1T": (_lowp(gate_perm_cols(_f32(inputs["Whh1"]).T)), LOWP),
        "W1T": (_lowp((_f32(inputs["fc_W1"])
                       * _f32(inputs["ln_g"])[None, :]).T), LOWP),  # [128, 64]
        "W2T": (_lowp(np.concatenate([_f32(inputs["fc_W2"]).T,
                                      _f32(inputs["fc_b2"])[None, :]],
                                     0)), LOWP),                   # [65, 3]
        "ones128": (_lowp(np.ones((128, 1))), LOWP),
        "ones13": (_lowp(np.ones((1, 3))), LOWP),
        "epsd": (np.full((128, 1), DIST_EPS, np.float32), F32),
        "epsl": (np.full((1, 1), LN_EPS, np.float32), F32),
    }
    for c in range(NC):
        consts[f"XW{c}"] = (_lowp(XW[c]), LOWP)

    consts["w1s"] = (_lowp(_f32((_f32(inputs["fc_W1"])
                                 * _f32(inputs["ln_g"])[None, :]).T
                                .sum(0, keepdims=True))), LOWP)    # [1, 64]

    pred0 = gpf_p[:, :3].mean(0).astype(np.float32)
    pred0c = pred0 - CEN
    init = {
        "prevaux0": (_lowp(np.concatenate(
            [np.broadcast_to(pred0c[:, None], (3, BL)),
             np.ones((1, BL), np.float32),
             np.zeros((28, BL), np.float32),
             np.broadcast_to((pred0c * pred0c)[:, None], (3, BL))],
            0)), LOWP),                                            # [35, BL]
        "prev30": (_lowp(np.broadcast_to(pred0[:, None], (3, BL))), LOWP),
        "h00": (_lowp(np.zeros((H, BL))), LOWP),
        "h10": (_lowp(np.zeros((H, BL))), LOWP),
        "c00": (np.zeros((H, BL), np.float32), F32),
        "c10": (np.zeros((H, BL), np.float32), F32),
        "q1re0": (_lowp(np.concatenate(
            [np.zeros((64, RSS * BL)), np.ones((1, RSS * BL))], 0)), LOWP),
        "xT0": (_lowp(np.r_[np.zeros((108, BL)), np.ones((1, BL))]), LOWP),
        "Are0": (_lowp(np.r_[np.zeros((64, BL)), np.ones((1, BL))]), LOWP),
    }
    # Pad the slot list so every 32-row band of every chunk is written:
    # unwritten PSUM rows would feed exp() with garbage (NaN via inf*0 in
    # the agg matmul). Dummy slots read KWT_e's zero columns -> scores 0,
    # u = exp(-0.5*ln(dummy d^2)) ~ 1e-6, and XW rows there are zero.
    used = {gb for (gb, _) in slot_of_group}
    slots_full = list(slot_of_group) + [(gb, 0) for gb in range(0, LP, 32)
                                        if gb not in used]
    meta = {
        "slots": slots_full,      # [(padded_base, group_r)] covers all LP
        "NC": NC,
        "LP": LP,
        # zero-bias fast paths: relu as a DVE max (no bias add needed)
        "b1q_zero": bool(not np.any(_f32(inputs["q_b1"])))
                    and not os.environ.get("KRELUACT"),
        "b1f_zero": bool(not np.any(_f32(inputs["fc_W1"]) @ _f32(inputs["ln_b"])
                                    + _f32(inputs["fc_b1"])))
                    and not os.environ.get("KRELUACT"),
    }
    return consts, init, meta


def _per_core_rss(rss_core):
    """rss_core [BL, T, RSS] -> rss_q [T, RSS*BL] and rssT [12, T*BL]."""
    rss_q = np.ascontiguousarray(
        rss_core.transpose(1, 2, 0).reshape(T, RSS * BL)).astype(NPLP)
    rssT = np.ascontiguousarray(
        rss_core.transpose(2, 1, 0).reshape(RSS, T * BL)).astype(NPLP)
    return rss_q, rssT


def build_nc(consts, init, meta, nsteps=T):
    """Build the per-core Bass program (SPMD across the 8 cores)."""
    nc = bacc.Bacc("TRN2", target_bir_lowering=False, debug=False,
                   num_devices=NCORES)
    R = RSS * BL  # 384
    NC = meta["NC"]
    slots = meta["slots"]

    dram = {}
    for k, (v, dt_) in {**consts, **init}.items():
        dram[k] = nc.dram_tensor(k, list(v.shape), dt_,
                                 kind="ExternalInput").ap()
    dram["rss_q"] = nc.dram_tensor("rss_q", [T, R], LOWP,
                                   kind="ExternalInput").ap()
    dram["rssT"] = nc.dram_tensor("rssT", [RSS, T * BL], LOWP,
                                  kind="ExternalInput").ap()
    d_out = nc.dram_tensor("out", [3, nsteps * BL], F32,
                           kind="ExternalOutput").ap()

    with tile.TileContext(nc) as tc:
        with (
            tc.tile_pool(name="const", bufs=1) as cpool,
            tc.tile_pool(name="state", bufs=1) as spool,
            tc.tile_pool(name="work", bufs=2) as wpool,
            tc.tile_pool(name="lnp", bufs=2) as lnpool,
            tc.tile_pool(name="qrow", bufs=3) as qpool,
            # PSUM: 8 banks total. q1(1) + scores(1) + dist/r3(1) + P(1)
            # + gates(3, cross-step prefetch) + small(1) = 8.
            tc.tile_pool(name="pq1", bufs=1, space="PSUM") as pq1,
            tc.tile_pool(name="psc", bufs=1, space="PSUM") as psc,
            tc.tile_pool(name="pds", bufs=1, space="PSUM") as pds,
            tc.tile_pool(name="pP", bufs=1, space="PSUM") as pP,
            tc.tile_pool(name="pg", bufs=3, space="PSUM") as pg,
            tc.tile_pool(name="pst", bufs=1, space="PSUM") as pst,
            tc.tile_pool(name="pbc", bufs=1, space="PSUM") as pbc,
        ):
            cs = {}
            for k, (v, dt_) in consts.items():
                t_ = cpool.tile(list(v.shape), dt_, tag=k, name=k)
                nc.sync.dma_start(t_[:], dram[k][:])
                cs[k] = t_
            t_rssT = cpool.tile([RSS, T * BL], LOWP, tag="rssT", name="t_rssT")
            nc.sync.dma_start(t_rssT[:], dram["rssT"][:])

            st = {}
            for k, shape, dt_ in [("prevaux", [35, BL], LOWP),
                                  ("prev3", [3, BL], LOWP),
                                  ("h0", [H, BL], LOWP),
                                  ("h1w", [H, 2 * BL], LOWP),
                                  ("c0", [H, BL], F32), ("c1", [H, BL], F32),
                                  ("q1re", [65, R], LOWP),
                                  ("xT", [109, BL], LOWP),
                                  ("Are", [65, BL], LOWP)]:
                st[k] = spool.tile(shape, dt_, tag=k, name="st_" + k)
            for k, src in [("prevaux", "prevaux0"), ("prev3", "prev30"),
                           ("h0", "h00"),
                           ("c0", "c00"), ("c1", "c10"),
                           ("q1re", "q1re0"), ("xT", "xT0"), ("Are", "Are0")]:
                nc.sync.dma_start(st[k][:], dram[src][:])
            nc.sync.dma_start(st["h1w"][:, 0:BL], dram["h10"][:])
            nc.sync.dma_start(st["h1w"][:, BL:2 * BL], dram["h10"][:])
            t_out = spool.tile([3, nsteps * BL], F32, tag="out_sb",
                               name="t_out")

            mm = nc.tensor.matmul
            act = nc.scalar.activation
            V = nc.vector

            def bc_r(ap3):
                """[3, BL] AP -> broadcast [3, RSS*BL] (free [[0,12],[1,BL]])."""
                return bass.AP(ap3.tensor, ap3.offset,
                               [ap3.ap[0], [0, RSS], ap3.ap[-1]])

            def gate_mms(gps, W, x, start, stop):
                # start=True ONLY on the first mm: the 0x1 flag clears
                # has_written BANK-WIDE, so a second start would flip the
                # already-written gate regions back to overwrite mode and
                # the later wih accumulation would silently drop them.
                for gi in range(4):
                    mm(gps[:, gi * BL:(gi + 1) * BL],
                       W[:, gi * H:(gi + 1) * H], x[:],
                       start=(start and gi == 0), stop=stop,
                       skip_group_check=True)

            PREFETCH = not os.environ.get("KNOPREFETCH")

            # ---- step-0 prefetches (the loop does these for t+1) ----
            qr_next = qpool.tile([1, R], LOWP, tag="qrow", name="qrow")
            nc.sync.dma_start(qr_next[:], dram["rss_q"][0:1, :])
            if PREFETCH:
                g0_next = pg.tile([128, 4 * BL], F32, tag="g", name="gps")
                gate_mms(g0_next, cs["Whh0T"], st["h0"], True, False)
                g1_next = pg.tile([128, 4 * BL], F32, tag="g", name="gps")
                gate_mms(g1_next, cs["Whh1T"], st["h1w"][:, 0:BL], True, False)

            for t in range(nsteps):
                prev = st["prevaux"][0:3, :] if t == 0 else \
                    t_out[:, (t - 1) * BL:t * BL]
                qrow = qr_next
                if PREFETCH:
                    gps0 = g0_next
                    gps1 = g1_next

                # xT rss rows: depends only on t; off the critical path on
                # the (otherwise idle) GpSimd engine
                GP = V if os.environ.get("KNOGPS") else nc.gpsimd
                GP.tensor_copy(st["xT"][96:108, :],
                               t_rssT[:, t * BL:(t + 1) * BL])

                # ---------- q-MLP + grouped scores ----------
                # q1a (rss half) emitted adjacent to q1b so the PSUM
                # accumulation group is textually contiguous (a cross-step
                # split group got its start flag mangled by the compiler);
                # the dep scheduler still hoists q1a early (needs only the
                # qrow DMA + the bank's WAR).
                q1ps = pq1.tile([64, R], F32, tag="q1", name="q1ps")
                mm(q1ps[:], cs["qW1a"][:], qrow[:],
                   start=True, stop=False, skip_group_check=True)
                mm(q1ps[:], cs["qW1b"][:], bc_r(st["prev3"][:]),
                   start=False, stop=True, skip_group_check=True)
                if meta["b1q_zero"]:
                    if os.environ.get("KRSPLIT"):
                        V.tensor_scalar_max(st["q1re"][0:64, 0:R // 2],
                                            q1ps[:, 0:R // 2], 0.0)
                        act(st["q1re"][0:64, R // 2:R], q1ps[:, R // 2:R],
                            AFT.Relu, bias=0.0)
                    else:
                        V.tensor_scalar_max(st["q1re"][0:64, :],
                                            q1ps[:], 0.0)
                else:
                    act(st["q1re"][0:64, :], q1ps[:], AFT.Relu,
                        bias=cs["b1q"][:, 0:1])

                # start=True ONLY on the first slot mm (bank-wide has_written
                # clear); the rest write fresh disjoint regions with 0x0
                # (has_written=false -> overwrite) and SET has_written, so
                # the -lnd matmul below accumulates everywhere.
                # NOTE: these column-tiled (tile_position) mms must keep
                # per-slot start=True/stop=True — a first-only-start scheme
                # (to let a later matmul accumulate -lnd into this bank)
                # corrupts the PSUM on HW.
                spsum = psc.tile([128, NC * BL], F32, tag="sc", name="spsum")
                for (gb, r) in slots:
                    c, lb = divmod(gb, 128)
                    mm(spsum[lb:lb + 32, c * BL:(c + 1) * BL],
                       cs["KWT_e"][:, gb:gb + 32],
                       st["q1re"][:, r * BL:(r + 1) * BL],
                       start=True, stop=True, tile_position=(0, lb))

                # ---------- distance term (fp32: cancellation) ----------
                dps = pds.tile([128, NC * BL], F32, tag="ds", name="dps")
                for c in range(NC):
                    sl = slice(c * 128, (c + 1) * 128)
                    mm(dps[:, c * BL:(c + 1) * BL], cs["poshT"][:, sl],
                       st["prevaux"][:, :], start=True, stop=True)

                if t + 1 < nsteps:
                    qr_next = qpool.tile([1, R], LOWP, tag="qrow",
                                         name="qrow")
                    nc.sync.dma_start(qr_next[:],
                                      dram["rss_q"][t + 1:t + 2, :])

                u_sb = wpool.tile([128, NC * BL], LOWP, tag="u", name="u_sb")
                lnd = wpool.tile([128, NC * BL], F32, tag="lnd", name="lnd")
                act(lnd[:], dps[:], AFT.Ln, bias=cs["epsd"][:, 0:1])
                diff = wpool.tile([128, NC * BL], F32, tag="diff",
                                  name="diff")
                V.tensor_sub(diff[:], spsum[:], lnd[:])
                act(u_sb[:], diff[:], AFT.Exp, scale=0.5)

                # ---------- aggregate + normalize ----------
                Pps = pP.tile([108, BL], F32, tag="P", name="Pps")
                for c in range(NC):
                    mm(Pps[:], cs[f"XW{c}"][:],
                       u_sb[:, c * BL:(c + 1) * BL],
                       start=(c == 0), stop=(c == NC - 1))

                # NOTE: reciprocal_approx_fast mis-reads PSUM operands
                # (BITWISE_NOT custom-DVE path); keep the exact iterative
                # reciprocal here — [12, 32] is cheap anyway. fp16 output
                # so the Ebc broadcast matmul runs 1-pass.
                rT = wpool.tile([RSS, BL], LOWP, tag="rT", name="rT")
                with nc.allow_low_precision(reason="softmax 1/sum -> fp16"):
                    V.reciprocal(rT[:], Pps[96:108, :])
                sbps = pst.tile([96, BL], F32, tag="small", name="sbps")
                mm(sbps[:], cs["Ebc"][:], rT[:], start=True, stop=True)
                sb96 = wpool.tile([96, BL], F32, tag="sb96", name="sb96")
                V.tensor_copy(sb96[:], sbps[:])
                V.tensor_mul(st["xT"][0:96, :], Pps[0:96, :], sb96[:])
                dbg_tiles = {"lnd": lnd, "diff": diff, "u": u_sb,
                             "sb96": sb96, "rT": rT}
                if os.environ.get("KXTLATE"):
                    V.tensor_copy(st["xT"][96:108, :],
                                  t_rssT[:, t * BL:(t + 1) * BL])

                # ---------- two LSTM layers ----------
                for ly in range(2):
                    wih = cs["Wih0T"] if ly == 0 else cs["Wih1T"]
                    xin = st["xT"] if ly == 0 else st["h0"]
                    cst = st["c0"] if ly == 0 else st["c1"]
                    if PREFETCH:
                        gps = gps0 if ly == 0 else gps1
                        gate_mms(gps, wih, xin, False, True)
                    else:
                        whh = cs["Whh0T"] if ly == 0 else cs["Whh1T"]
                        hin = st["h0"] if ly == 0 else st["h1w"][:, 0:BL]
                        gps = pg.tile([128, 4 * BL], F32, tag="g",
                                      name="gps")
                        gate_mms(gps, whh, hin, True, False)
                        gate_mms(gps, wih, xin, False, True)

                    # i,f,g sigmoids feed the cell update; o's sigmoid is
                    # computed while exp(-2c) sits on the ACT engine.
                    e0 = wpool.tile([128, 3 * BL], F32, tag=f"e{ly}",
                                    name="e0t")
                    act(e0[:], gps[:, 0:3 * BL], AFT.Exp, scale=-1.0)
                    eo = wpool.tile([128, BL], F32, tag=f"eo{ly}", name="eot")
                    act(eo[:], gps[:, 3 * BL:4 * BL], AFT.Exp, scale=-1.0)
                    ea = wpool.tile([128, 3 * BL], F32, tag=f"ea{ly}",
                                    name="eat")
                    V.tensor_scalar_add(ea[:], e0[:], 1.0)
                    rg = wpool.tile([128, 3 * BL], F32, tag=f"rg{ly}",
                                    name="rgt")
                    V.reciprocal_approx_fast(rg[:], ea[:])
                    tg = wpool.tile([128, BL], F32, tag=f"tg{ly}", name="tgt")
                    V.tensor_scalar(tg[:], rg[:, 2 * BL:3 * BL], 2.0, 1.0,
                                    op0=ALU.mult, op1=ALU.subtract)
                    p_ = wpool.tile([128, BL], F32, tag=f"p{ly}", name="p_t")
                    V.tensor_mul(p_[:], rg[:, BL:2 * BL], cst[:])  # sig(f)*c
                    q_ = wpool.tile([128, BL], F32, tag=f"q{ly}", name="q_t")
                    V.tensor_mul(q_[:], rg[:, 0:BL], tg[:])        # sig(i)*tg
                    V.tensor_add(cst[:], p_[:], q_[:])

                    ec = wpool.tile([128, BL], F32, tag=f"ec{ly}", name="ect")
                    act(ec[:], cst[:], AFT.Exp, scale=-2.0)
                    # o-gate sigmoid on DVE during the exp(-2c) latency
                    eao = wpool.tile([128, BL], F32, tag=f"eao{ly}",
                                     name="eaot")
                    V.tensor_scalar_add(eao[:], eo[:], 1.0)
                    ro = wpool.tile([128, BL], F32, tag=f"ro{ly}", name="rot")
                    V.reciprocal_approx_fast(ro[:], eao[:])
                    eac = wpool.tile([128, BL], F32, tag=f"eac{ly}",
                                     name="eact")
                    V.tensor_scalar_add(eac[:], ec[:], 1.0)
                    rc = wpool.tile([128, BL], F32, tag=f"rc{ly}", name="rct")
                    V.reciprocal_approx_fast(rc[:], eac[:])
                    thc = wpool.tile([128, BL], F32, tag=f"thc{ly}",
                                     name="thct")
                    V.tensor_scalar(thc[:], rc[:], 2.0, 1.0,
                                    op0=ALU.mult, op1=ALU.subtract)
                    hout = st["h0"][:] if ly == 0 else st["h1w"][:, 0:BL]
                    V.tensor_mul(hout, ro[:], thc[:])

                    # prefetch next step's Whh gates the moment h is out
                    if PREFETCH and ly == 0 and t + 1 < nsteps:
                        g0_next = pg.tile([128, 4 * BL], F32, tag="g",
                                          name="gps")
                        gate_mms(g0_next, cs["Whh0T"], st["h0"], True, False)

                # ---------- LayerNorm + fc head ----------
                # stat split: the sum(h) half issues right at h1 so the
                # -mu/m2/vv/rstd pole starts ~350ns earlier; sum(h^2)
                # follows the h^2 multiply. Emitted BEFORE the whh1
                # prefetch so the in-order PE doesn't queue 4 gate
                # matmuls ahead of them.
                stat = pst.tile([1, 2 * BL], F32, tag="small", name="stat")
                mm(stat[0:1, 0:BL], cs["ones128"][:], st["h1w"][:, 0:BL],
                   start=True, stop=True)
                V.tensor_mul(st["h1w"][:, BL:2 * BL], st["h1w"][:, 0:BL],
                             st["h1w"][:, 0:BL])
                mm(stat[0:1, BL:2 * BL], cs["ones128"][:],
                   st["h1w"][:, BL:2 * BL], start=True, stop=True)
                if PREFETCH and t + 1 < nsteps:
                    g1_next = pg.tile([128, 4 * BL], F32, tag="g",
                                      name="gps")
                    gate_mms(g1_next, cs["Whh1T"], st["h1w"][:, 0:BL],
                             True, False)

                bsrc = lnpool.tile([1, 2 * BL], LOWP, tag="bsrc", name="bsrc")
                V.tensor_scalar_mul(bsrc[0:1, 0:BL], stat[0:1, 0:BL],
                                    -1.0 / H)  # -mu (fp16 is plenty)
                m2 = lnpool.tile([1, BL], F32, tag="m2", name="m2")
                V.tensor_mul(m2[:], bsrc[0:1, 0:BL], bsrc[0:1, 0:BL])
                vv = lnpool.tile([1, BL], F32, tag="vv", name="vv")
                V.scalar_tensor_tensor(vv[:], stat[0:1, BL:2 * BL], 1.0 / H,
                                       m2[:], op0=ALU.mult, op1=ALU.subtract)
                lv = lnpool.tile([1, BL], F32, tag="lv", name="lv")
                act(lv[:], vv[:], AFT.Ln, bias=cs["epsl"][:, 0:1])
                act(bsrc[0:1, BL:2 * BL], lv[:], AFT.Exp, scale=-0.5)

                # relu(rstd*(W1g@(h1-mu))) = rstd*relu(W1g@h1 - mu*w1s):
                # centering is a K=1 accumulating matmul; rstd scales the
                # (bias-free) head output at the very end.
                a2ps = pst.tile([64, BL], F32, tag="small", name="a2ps")
                mm(a2ps[:], cs["W1T"][:], st["h1w"][:, 0:BL],
                   start=True, stop=False)
                mm(a2ps[:], cs["w1s"][:], bsrc[0:1, 0:BL],
                   start=False, stop=True)
                if meta["b1f_zero"]:
                    V.tensor_scalar_max(st["Are"][0:64, :], a2ps[:], 0.0)
                else:
                    act(st["Are"][0:64, :], a2ps[:], AFT.Relu,
                        bias=cs["b1f"][:, 0:1])
                prps = pst.tile([3, BL], F32, tag="small", name="prps")
                mm(prps[:], cs["W2T"][:], st["Are"][:], start=True, stop=True)
                # r3 broadcast reuses the dist PSUM bank (free this late
                # in the step; next dist write waits for r3sb's read)
                if os.environ.get("KR3SEP"):
                    r3ps = pbc.tile([3, BL], F32, tag="bc", name="r3ps")[:]
                else:
                    r3ps = dps[0:3, 0:BL]
                mm(r3ps, cs["ones13"][:], bsrc[0:1, BL:2 * BL],
                   start=True, stop=True)
                r3sb = lnpool.tile([3, BL], F32, tag="r3sb", name="r3sb")
                V.tensor_copy(r3sb[:], r3ps)
                nxt = t_out[:, t * BL:(t + 1) * BL]
                if t + 1 < nsteps:
                    V.tensor_mul(nxt, prps[:], r3sb[:])
                    if os.environ.get("KPREV3CAST"):
                        V.tensor_copy(st["prev3"][:], nxt)
                    else:
                        V.tensor_mul(st["prev3"][:], prps[:], r3sb[:])
                    V.tensor_scalar_add(st["prevaux"][0:3, :], nxt, -CEN)
                    V.tensor_mul(st["prevaux"][32:35, :],
                                 st["prevaux"][0:3, :],
                                 st["prevaux"][0:3, :])
                else:
                    V.tensor_mul(nxt, prps[:], r3sb[:])
                    q1_next = pq1.tile([64, R], F32, tag="q1", name="q1ps")
                    mm(q1_next[:], cs["qW1a"][:], qr_next[:],
                       start=True, stop=False, skip_group_check=True)

            if os.environ.get("KDBG"):
                for nm, ap, shp, dt_ in [
                        ("d_lnd", dbg_tiles["lnd"], [128, NC * BL], F32),
                        ("d_diff", dbg_tiles["diff"], [128, NC * BL], F32),
                        ("d_u", dbg_tiles["u"], [128, NC * BL], LOWP),
                        ("d_sb96", dbg_tiles["sb96"], [96, BL], F32),
                        ("d_rT", dbg_tiles["rT"], [RSS, BL], LOWP),
                        ("d_prevaux", st["prevaux"], [35, BL], LOWP),
                        ("d_q1re", st["q1re"], [65, R], LOWP),
                        ("d_xT", st["xT"], [109, BL], LOWP),
                        ("d_h0", st["h0"], [H, BL], LOWP),
                        ("d_c0", st["c0"], [H, BL], F32),
                        ("d_h1w", st["h1w"], [H, 2 * BL], LOWP),
                        ("d_Are", st["Are"], [65, BL], LOWP),
                        ("d_prev3", st["prev3"], [3, BL], LOWP)]:
                    dd = nc.dram_tensor(nm, shp, dt_,
                                        kind="ExternalOutput").ap()
                    nc.sync.dma_start(dd[:], ap[:])
            nc.sync.dma_start(d_out[:], t_out[:])

    nc.compile()
    return nc


def make_in_maps(consts, init, rss_seq):
    base = {k: v for k, (v, _) in {**consts, **init}.items()}
    in_maps = []
    for k in range(NCORES):
        rss_q, rssT = _per_core_rss(rss_seq[k * BL:(k + 1) * BL])
        m = dict(base)
        m["rss_q"] = rss_q
        m["rssT"] = rssT
        in_maps.append(m)
    return in_maps


def kernel(**inputs):
    rss_seq = _f32(inputs["rss_seq"])
    consts, init, meta = _host_prep(inputs)
    nc = build_nc(consts, init, meta, nsteps=T)
    in_maps = make_in_maps(consts, init, rss_seq)
    res = run_bass_kernel_spmd(nc, in_maps, core_ids=list(range(NCORES)))
    outs = []
    for k in range(NCORES):
        o = res.results[k]["out"]
        outs.append(np.asarray(o).reshape(3, T, BL).transpose(2, 1, 0))
    return np.ascontiguousarray(np.concatenate(outs, 0))


# revision 69
# speedup vs baseline: 1.0146x; 1.0090x over previous
"""Trainium2 Bass kernel for nn_Attentive_VLP_LSTM.

kernel(**inputs) takes the FULL unsharded inputs (numpy) and returns the
FULL [B, T, 3] output. Batch is sharded over 8 NeuronCores (32 rows
each); each core runs a fully-unrolled T=256 recurrent Bass/Tile program.

Design notes (v3 — fp16 matmuls + software pipelining):
  - All matmuls run in fp16 (1 PE pass @ 1 cyc/row vs fp32's 2 passes @
    4 cyc/row; fp16 over bf16 for the 8x finer mantissa at equal speed).
    The distance matmul survives fp16 by recentering all positions by
    CEN=2.5 (shrinks the |led|^2-scale terms whose cancellation dominates
    the quantization error of d^2).
  - Single ACT table set (natural_log_exp_and_others): softmax uses
    exp(q.k - 0.5*ln(d^2+eps)); LayerNorm rstd = exp(-0.5*ln(var+eps));
    LSTM sigmoid/tanh built from exp + DVE reciprocal_approx_fast.
    Gate layout is (i,f,g,o): the o-gate sigmoid runs on DVE during the
    exp(-2c) ACT latency. Relu runs as a DVE max when biases are zero.
  - Cross-step software pipelining: the Whh gate matmuls for step t+1
    are issued as soon as h(t) is ready (they hide inside step t's
    DVE/ACT phase). The framework list-schedules by dependencies, so the
    rss half of the q-MLP hoists itself off the critical path.
  - PSUM accumulation-group pitfalls (hardware-verified):
    * start=True clears has_written beyond the instruction's own write
      region - a multi-region group must put start=True ONLY on its
      first matmul, or later accumulating writes silently overwrite.
    * a group whose start/stop halves are textually far apart (emitted
      across the step boundary) can lose its start flag entirely in
      compilation; keep groups contiguous in emission order.
    * DVE/custom-DVE ops must not read two PSUM operands, and
      reciprocal_approx_fast mis-reads PSUM inputs outright.
  - LEDs are sorted by frequency and laid out in 32-aligned slots per
    frequency group (matmul tile_position needs 32-aligned bases); the
    per-(batch, rss-row) softmax runs only over its own group; all 16
    slots are emitted (dummies hit KWT_e's zero columns) so no PSUM row
    feeds exp() uninitialized. Group sums + led_feat aggregation are one
    block-diagonal matmul (XW).
  - Activations are feature-major [feat, batch]; biases are folded into
    matmuls via ones-rows. Output accumulates in SBUF [3, T*32], host
    transposes. r3 broadcast PSUM shares the dist bank (disjoint in
    time) so everything fits in 8 PSUM banks.
"""

import os
import sys

import numpy as np

for _p in ("/opt/trn_rl_repo", "/root/.axon_site", "/root/.axon_site/_ro/pypackages"):
    if _p not in sys.path and os.path.isdir(_p):
        sys.path.append(_p)

import ml_dtypes

import concourse.bass as bass
import concourse.tile as tile
from concourse import bacc, mybir
from concourse.bass_utils import run_bass_kernel_spmd

# The act-table chooser assigns each activation the FIRST table set that
# contains its function, which thrashes between `natural_log` and
# `exp_and_others` (one ~1.3us reload per switch, several per step).
# Every ACT function this kernel uses (Exp, Ln, Relu) lives in
# `natural_log_exp_and_others`, so blank out every other set (order and
# indices preserved -> act_func_set_id stays correct) to get ONE load.
_KEEP_ACT_SET = "natural_log_exp_and_others"
_orig_get_act_tables = bacc.get_activation_tables


def _patched_get_act_tables(arch):
    tabs = _orig_get_act_tables(arch)
    return {name: (fns if name == _KEEP_ACT_SET else set())
            for name, fns in tabs.items()}


bacc.get_activation_tables = _patched_get_act_tables

B, T, RSS, L, FEAT, H = 256, 256, 12, 256, 8, 128
NCORES = 8
BL = B // NCORES  # 32 batch rows per core
AFT = mybir.ActivationFunctionType
ALU = mybir.AluOpType
F32 = mybir.dt.float32
LOWP = mybir.dt.float16   # fp16: 1 cyc/row on PE like bf16, 8x finer mantissa
NPLP = np.float16
if os.environ.get("KF32"):        # debug: full fp32 matmuls
    LOWP = mybir.dt.float32
    NPLP = np.float32
DIST_EPS = 1e-8
LN_EPS = 1e-5
CEN = 2.5  # room-center shift for the fp16 distance matmul

# Gate column-block order in the [128, 4*BL] gate psum: i, f, g, o
# (torch weight-row order, kept as-is). g pre-doubled for the tanh fold;
# o last so its sigmoid chain can overlap the exp(-2c) ACT latency.
_GATE_ROWS = [(0, H), (H, 2 * H), (2 * H, 3 * H), (3 * H, 4 * H)]


def _f32(x):
    return np.ascontiguousarray(np.asarray(x, np.float32))


def _lowp(x):
    return np.ascontiguousarray(np.asarray(x, NPLP))


def _host_prep(inputs):
    """All static marshalling: LED sort + 32-aligned slot layout, small
    one-time MLPs, weight folds."""
    gpf = _f32(inputs["global_led_pos_freq"])  # [L, 4]
    freq = gpf[:, 3]
    perm = np.argsort(freq, kind="stable")
    gpf_p = gpf[perm]

    relu = lambda x: np.maximum(x, np.float32(0))
    lin = lambda x, W, bb: x @ _f32(W).T + _f32(bb)

    led_feat = relu(lin(relu(lin(gpf_p, inputs["enc_W1"], inputs["enc_b1"])),
                        inputs["enc_W2"], inputs["enc_b2"]))  # [L, 8]
    keys = lin(relu(lin(np.concatenate([led_feat, gpf_p[:, :3]], 1),
                        inputs["k_W1"], inputs["k_b1"])),
               inputs["k_W2"], inputs["k_b2"])  # [L, 64]

    # --- padded slot layout: each freq group starts at a 32-boundary ---
    freq_p = gpf_p[:, 3]
    bounds = np.searchsorted(freq_p, np.arange(1, RSS + 2) - 0.5)
    slot_of_group = []   # list of (slot_base, r) 32-wide scores-MM slots
    src_list = []
    base = 0
    for r in range(RSS):
        g0, g1 = int(bounds[r]), int(bounds[r + 1])
        n = g1 - g0
        nslots = max(1, (n + 31) // 32)
        for s in range(nslots):
            slot_of_group.append((base + 32 * s, r))
        sl = -np.ones(nslots * 32, np.int64)
        sl[:n] = np.arange(g0, g1)
        src_list.append(sl)
        base += nslots * 32
    pad_src = np.concatenate(src_list)  # padded-pos -> sorted-led idx or -1
    LP = ((base + 127) // 128) * 128    # padded led count, 128-multiple
    pad_src = np.concatenate([pad_src, -np.ones(LP - base, np.int64)])
    NC = LP // 128
    real = pad_src >= 0

    def expand(arr_p, fill=0.0):
        """[L, ...] sorted-led array -> [LP, ...] padded."""
        out = np.full((LP,) + arr_p.shape[1:], fill, np.float32)
        out[real] = arr_p[pad_src[real]]
        return out

    led_feat_e = expand(led_feat)
    keys_e = expand(keys)
    pos_e = expand(gpf_p[:, :3], fill=100.0)  # dummies far away (fp16-safe)
    r_of_e = np.full(LP, 0, np.int64)
    r_of_e[real] = np.rint(freq_p[pad_src[real]] - 1.0).astype(np.int64)

    KW = 2.0 * keys_e @ _f32(inputs["q_W2"])      # [LP, 64]
    kb2 = 2.0 * keys_e @ _f32(inputs["q_b2"])     # [LP]
    KWT_e = _f32(np.concatenate([KW.T, kb2[None, :]], 0))  # [65, LP]

    # prevaux rows: 0:3 pred-CEN, 3 ones, 4:32 zero pad, 32:35 (pred-CEN)^2
    # (compute-engine writes must start at 32-aligned partitions).
    # Positions are recentered by CEN so the d^2 matmul is fp16-safe:
    # |led|^2-scale terms shrink ~4x, keeping the catastrophic-cancellation
    # error of quantized inputs well below ln()'s sensitivity.
    pos_c = pos_e - CEN
    poshT = np.zeros((35, LP), np.float32)
    poshT[0:3] = -2.0 * pos_c.T
    poshT[3] = (pos_c * pos_c).sum(1)
    poshT[32:35] = 1.0

    # XW [NC][128, 108]: cols 0:96 = agg (r*8+j), 96:108 = group sums
    XW = np.zeros((NC, 128, 108), np.float32)
    for lp in range(LP):
        if not real[lp]:
            continue
        c, lr = divmod(lp, 128)
        r = r_of_e[lp]
        XW[c, lr, r * 8:(r + 1) * 8] = led_feat_e[lp]
        XW[c, lr, 96 + r] = 1.0

    Ebc = np.zeros((RSS, 96), np.float32)
    for r in range(RSS):
        Ebc[r, r * 8:(r + 1) * 8] = 1.0

    def gate_perm_cols(Wt):  # [K, 512] torch-order cols; g-gate 2x fold
        blocks = [Wt[:, a:b] for (a, b) in _GATE_ROWS]
        blocks[2] = 2.0 * blocks[2]  # tanh(g) from exp(-2g)
        return _f32(np.concatenate(blocks, 1))

    def wih_feature_rows(Wih):  # [512, 108] -> feature-major [108, 512]
        Wt = _f32(Wih).T
        out = np.zeros((108, 4 * H), np.float32)
        for r in range(RSS):
            out[96 + r] = Wt[r * 9]
            for j in range(FEAT):
                out[r * 8 + j] = Wt[r * 9 + 1 + j]
        return out

    bsum0 = _f32(inputs["bih0"]) + _f32(inputs["bhh0"])
    bsum1 = _f32(inputs["bih1"]) + _f32(inputs["bhh1"])
    Wih0T = np.concatenate([wih_feature_rows(inputs["Wih0"]),
                            bsum0[None, :]], 0)  # [109, 512]

    # layer-1 bias via exp(-(v+b)) = exp(-v)*exp(-b); g block pre-doubled
    eb1 = np.zeros((H, 4), np.float32)
    for gi, (a, b_) in enumerate(_GATE_ROWS):
        s = -1.0 if gi < 3 else -2.0
        eb1[:, gi] = np.exp(s * bsum1[a:b_])

    # consts: name -> (array, dtype). bf16 for every matmul operand
    # except the fp32 dist path (poshT) + DVE-side scalars.
    consts = {
        "qW1a": (_lowp(_f32(inputs["q_W1"]).T[0:1]), LOWP),       # [1, 64]
        "qW1b": (_lowp(_f32(inputs["q_W1"]).T[1:4]), LOWP),       # [3, 64]
        "b1q": (_f32(inputs["q_b1"])[:, None], F32),              # [64, 1]
        "b1f": ((_f32(inputs["fc_W1"]) @ _f32(inputs["ln_b"])
                 + _f32(inputs["fc_b1"]))[:, None], F32),          # [64, 1]
        "KWT_e": (_lowp(KWT_e), LOWP),                            # [65, LP]
        "poshT": (_lowp(poshT), LOWP),                            # [35, LP]
        "Ebc": (_lowp(Ebc), LOWP),                                # [12, 96]
        "Wih0T": (_lowp(gate_perm_cols(Wih0T)), LOWP),            # [109, 512]
        "Whh0T": (_lowp(gate_perm_cols(_f32(inputs["Whh0"]).T)), LOWP),
        "Wih1T": (_lowp(gate_perm_cols(_f32(inputs["Wih1"]).T)), LOWP),
        "Whh1T": (_lowp(gate_perm_cols(_f32(inputs["Whh1"]).T)), LOWP),
        "W1T": (_lowp((_f32(inputs["fc_W1"])
                       * _f32(inputs["ln_g"])[None, :]).T), LOWP),  # [128, 64]
        "W2T": (_lowp(np.concatenate([_f32(inputs["fc_W2"]).T,
                                      _f32(inputs["fc_b2"])[None, :]],
                                     0)), LOWP),                   # [65, 3]
        "ones128": (_lowp(np.ones((128, 1))), LOWP),
        "ones13": (_lowp(np.ones((1, 3))), LOWP),
        "epsd": (np.full((128, 1), DIST_EPS, np.float32), F32),
        "epsl": (np.full((1, 1), LN_EPS, np.float32), F32),
    }
    for c in range(NC):
        consts[f"XW{c}"] = (_lowp(XW[c]), LOWP)

    consts["w1s"] = (_lowp(_f32((_f32(inputs["fc_W1"])
                                 * _f32(inputs["ln_g"])[None, :]).T
                                .sum(0, keepdims=True))), LOWP)    # [1, 64]

    pred0 = gpf_p[:, :3].mean(0).astype(np.float32)
    pred0c = pred0 - CEN
    init = {
        "prevaux0": (_lowp(np.concatenate(
            [np.broadcast_to(pred0c[:, None], (3, BL)),
             np.ones((1, BL), np.float32),
             np.zeros((28, BL), np.float32),
             np.broadcast_to((pred0c * pred0c)[:, None], (3, BL))],
            0)), LOWP),                                            # [35, BL]
        "prev30": (_lowp(np.broadcast_to(pred0[:, None], (3, BL))), LOWP),
        "h00": (_lowp(np.zeros((H, BL))), LOWP),
        "h10": (_lowp(np.zeros((H, BL))), LOWP),
        "c00": (np.zeros((H, BL), np.float32), F32),
        "c10": (np.zeros((H, BL), np.float32), F32),
        "q1re0": (_lowp(np.concatenate(
            [np.zeros((64, RSS * BL)), np.ones((1, RSS * BL))], 0)), LOWP),
        "xT0": (_lowp(np.r_[np.zeros((108, BL)), np.ones((1, BL))]), LOWP),
        "Are0": (_lowp(np.r_[np.zeros((64, BL)), np.ones((1, BL))]), LOWP),
    }
    # Pad the slot list so every 32-row band of every chunk is written:
    # unwritten PSUM rows would feed exp() with garbage (NaN via inf*0 in
    # the agg matmul). Dummy slots read KWT_e's zero columns -> scores 0,
    # u = exp(-0.5*ln(dummy d^2)) ~ 1e-6, and XW rows there are zero.
    used = {gb for (gb, _) in slot_of_group}
    slots_full = list(slot_of_group) + [(gb, 0) for gb in range(0, LP, 32)
                                        if gb not in used]
    meta = {
        "slots": slots_full,      # [(padded_base, group_r)] covers all LP
        "NC": NC,
        "LP": LP,
        # zero-bias fast paths: relu as a DVE max (no bias add needed)
        "b1q_zero": bool(not np.any(_f32(inputs["q_b1"])))
                    and not os.environ.get("KRELUACT"),
        "b1f_zero": bool(not np.any(_f32(inputs["fc_W1"]) @ _f32(inputs["ln_b"])
                                    + _f32(inputs["fc_b1"])))
                    and not os.environ.get("KRELUACT"),
    }
    return consts, init, meta


def _per_core_rss(rss_core):
    """rss_core [BL, T, RSS] -> rss_q [T, RSS*BL] and rssT [12, T*BL]."""
    rss_q = np.ascontiguousarray(
        rss_core.transpose(1, 2, 0).reshape(T, RSS * BL)).astype(NPLP)
    rssT = np.ascontiguousarray(
        rss_core.transpose(2, 1, 0).reshape(RSS, T * BL)).astype(NPLP)
    return rss_q, rssT


def build_nc(consts, init, meta, nsteps=T):
    """Build the per-core Bass program (SPMD across the 8 cores)."""
    nc = bacc.Bacc("TRN2", target_bir_lowering=False, debug=False,
                   num_devices=NCORES)
    R = RSS * BL  # 384
    NC = meta["NC"]
    slots = meta["slots"]

    dram = {}
    for k, (v, dt_) in {**consts, **init}.items():
        dram[k] = nc.dram_tensor(k, list(v.shape), dt_,
                                 kind="ExternalInput").ap()
    dram["rss_q"] = nc.dram_tensor("rss_q", [T, R], LOWP,
                                   kind="ExternalInput").ap()
    dram["rssT"] = nc.dram_tensor("rssT", [RSS, T * BL], LOWP,
                                  kind="ExternalInput").ap()
    d_out = nc.dram_tensor("out", [3, nsteps * BL], F32,
                           kind="ExternalOutput").ap()

    with tile.TileContext(nc) as tc:
        with (
            tc.tile_pool(name="const", bufs=1) as cpool,
            tc.tile_pool(name="state", bufs=1) as spool,
            tc.tile_pool(name="work", bufs=2) as wpool,
            tc.tile_pool(name="lnp", bufs=2) as lnpool,
            tc.tile_pool(name="qrow", bufs=3) as qpool,
            # PSUM: 8 banks total. q1(1) + scores(1) + dist/r3(1) + P(1)
            # + gates(3, cross-step prefetch) + small(1) = 8.
            tc.tile_pool(name="pq1", bufs=1, space="PSUM") as pq1,
            tc.tile_pool(name="psc", bufs=1, space="PSUM") as psc,
            tc.tile_pool(name="pds", bufs=1, space="PSUM") as pds,
            tc.tile_pool(name="pP", bufs=1, space="PSUM") as pP,
            tc.tile_pool(name="pg", bufs=3, space="PSUM") as pg,
            tc.tile_pool(name="pst", bufs=1, space="PSUM") as pst,
            tc.tile_pool(name="pbc", bufs=1, space="PSUM") as pbc,
        ):
            cs = {}
            for k, (v, dt_) in consts.items():
                t_ = cpool.tile(list(v.shape), dt_, tag=k, name=k)
                nc.sync.dma_start(t_[:], dram[k][:])
                cs[k] = t_
            t_rssT = cpool.tile([RSS, T * BL], LOWP, tag="rssT", name="t_rssT")
            nc.sync.dma_start(t_rssT[:], dram["rssT"][:])

            st = {}
            for k, shape, dt_ in [("prevaux", [35, BL], LOWP),
                                  ("prev3", [3, BL], LOWP),
                                  ("h0", [H, BL], LOWP),
                                  ("h1w", [H, 2 * BL], LOWP),
                                  ("c0", [H, BL], F32), ("c1", [H, BL], F32),
                                  ("q1re", [65, R], LOWP),
                                  ("xT", [109, BL], LOWP),
                                  ("Are", [65, BL], LOWP)]:
                st[k] = spool.tile(shape, dt_, tag=k, name="st_" + k)
            for k, src in [("prevaux", "prevaux0"), ("prev3", "prev30"),
                           ("h0", "h00"),
                           ("c0", "c00"), ("c1", "c10"),
                           ("q1re", "q1re0"), ("xT", "xT0"), ("Are", "Are0")]:
                nc.sync.dma_start(st[k][:], dram[src][:])
            nc.sync.dma_start(st["h1w"][:, 0:BL], dram["h10"][:])
            nc.sync.dma_start(st["h1w"][:, BL:2 * BL], dram["h10"][:])
            t_out = spool.tile([3, nsteps * BL], F32, tag="out_sb",
                               name="t_out")

            mm = nc.tensor.matmul
            act = nc.scalar.activation
            V = nc.vector

            def bc_r(ap3):
                """[3, BL] AP -> broadcast [3, RSS*BL] (free [[0,12],[1,BL]])."""
                return bass.AP(ap3.tensor, ap3.offset,
                               [ap3.ap[0], [0, RSS], ap3.ap[-1]])

            def gate_mms(gps, W, x, start, stop):
                # start=True ONLY on the first mm: the 0x1 flag clears
                # has_written BANK-WIDE, so a second start would flip the
                # already-written gate regions back to overwrite mode and
                # the later wih accumulation would silently drop them.
                for gi in range(4):
                    mm(gps[:, gi * BL:(gi + 1) * BL],
                       W[:, gi * H:(gi + 1) * H], x[:],
                       start=(start and gi == 0), stop=stop,
                       skip_group_check=True)

            PREFETCH = not os.environ.get("KNOPREFETCH")

            # ---- step-0 prefetches (the loop does these for t+1) ----
            qr_next = qpool.tile([1, R], LOWP, tag="qrow", name="qrow")
            nc.sync.dma_start(qr_next[:], dram["rss_q"][0:1, :])
            if PREFETCH:
                g0_next = pg.tile([128, 4 * BL], F32, tag="g", name="gps")
                gate_mms(g0_next, cs["Whh0T"], st["h0"], True, False)
                g1_next = pg.tile([128, 4 * BL], F32, tag="g", name="gps")
                gate_mms(g1_next, cs["Whh1T"], st["h1w"][:, 0:BL], True, False)

            for t in range(nsteps):
                prev = st["prevaux"][0:3, :] if t == 0 else \
                    t_out[:, (t - 1) * BL:t * BL]
                qrow = qr_next
                if PREFETCH:
                    gps0 = g0_next
                    gps1 = g1_next

                # xT rss rows: depends only on t; off the critical path on
                # the (otherwise idle) GpSimd engine
                GP = V if os.environ.get("KNOGPS") else nc.gpsimd
                GP.tensor_copy(st["xT"][96:108, :],
                               t_rssT[:, t * BL:(t + 1) * BL])

                # ---------- q-MLP + grouped scores ----------
                # q1a (rss half) emitted adjacent to q1b so the PSUM
                # accumulation group is textually contiguous (a cross-step
                # split group got its start flag mangled by the compiler);
                # the dep scheduler still hoists q1a early (needs only the
                # qrow DMA + the bank's WAR).
                q1ps = pq1.tile([64, R], F32, tag="q1", name="q1ps")
                mm(q1ps[:], cs["qW1a"][:], qrow[:],
                   start=True, stop=False, skip_group_check=True)
                mm(q1ps[:], cs["qW1b"][:], bc_r(st["prev3"][:]),
                   start=False, stop=True, skip_group_check=True)
                if meta["b1q_zero"]:
                    if os.environ.get("KRSPLIT"):
                        V.tensor_scalar_max(st["q1re"][0:64, 0:R // 2],
                                            q1ps[:, 0:R // 2], 0.0)
                        act(st["q1re"][0:64, R // 2:R], q1ps[:, R // 2:R],
                            AFT.Relu, bias=0.0)
                    else:
                        V.tensor_scalar_max(st["q1re"][0:64, :],
                                            q1ps[:], 0.0)
                else:
                    act(st["q1re"][0:64, :], q1ps[:], AFT.Relu,
                        bias=cs["b1q"][:, 0:1])

                # start=True ONLY on the first slot mm (bank-wide has_written
                # clear); the rest write fresh disjoint regions with 0x0
                # (has_written=false -> overwrite) and SET has_written, so
                # the -lnd matmul below accumulates everywhere.
                # NOTE: these column-tiled (tile_position) mms must keep
                # per-slot start=True/stop=True — a first-only-start scheme
                # (to let a later matmul accumulate -lnd into this bank)
                # corrupts the PSUM on HW.
                spsum = psc.tile([128, NC * BL], F32, tag="sc", name="spsum")
                for (gb, r) in slots:
                    c, lb = divmod(gb, 128)
                    mm(spsum[lb:lb + 32, c * BL:(c + 1) * BL],
                       cs["KWT_e"][:, gb:gb + 32],
                       st["q1re"][:, r * BL:(r + 1) * BL],
                       start=True, stop=True, tile_position=(0, lb))

                # ---------- distance term (fp32: cancellation) ----------
                dps = pds.tile([128, NC * BL], F32, tag="ds", name="dps")
                for c in range(NC):
                    sl = slice(c * 128, (c + 1) * 128)
                    mm(dps[:, c * BL:(c + 1) * BL], cs["poshT"][:, sl],
                       st["prevaux"][:, :], start=True, stop=True)

                if t + 1 < nsteps:
                    qr_next = qpool.tile([1, R], LOWP, tag="qrow",
                                         name="qrow")
                    nc.sync.dma_start(qr_next[:],
                                      dram["rss_q"][t + 1:t + 2, :])

                # u = exp(0.5*spsum) * exp(-0.5*ln(d^2+eps)): the distance
                # factor rd finishes on ACT while the score matmuls still
                # run, so the post-scores join is just exp + one multiply
                # (the old spsum-lnd DVE subtract sat fully on the path).
                # Both factors fp32 (each can exceed fp16 range alone).
                u_sb = wpool.tile([128, NC * BL], LOWP, tag="u", name="u_sb")
                lnd = wpool.tile([128, NC * BL], F32, tag="lnd", name="lnd")
                act(lnd[:], dps[:], AFT.Ln, bias=cs["epsd"][:, 0:1])
                rd = wpool.tile([128, NC * BL], F32, tag="rd", name="rd")
                act(rd[:], lnd[:], AFT.Exp, scale=-0.5)
                es = wpool.tile([128, NC * BL], F32, tag="es", name="es")
                act(es[:], spsum[:], AFT.Exp, scale=0.5)
                V.tensor_mul(u_sb[:], es[:], rd[:])

                # ---------- aggregate + normalize ----------
                Pps = pP.tile([108, BL], F32, tag="P", name="Pps")
                for c in range(NC):
                    mm(Pps[:], cs[f"XW{c}"][:],
                       u_sb[:, c * BL:(c + 1) * BL],
                       start=(c == 0), stop=(c == NC - 1))

                # NOTE: reciprocal_approx_fast mis-reads PSUM operands
                # (BITWISE_NOT custom-DVE path); keep the exact iterative
                # reciprocal here — [12, 32] is cheap anyway. fp16 output
                # so the Ebc broadcast matmul runs 1-pass.
                rT = wpool.tile([RSS, BL], LOWP, tag="rT", name="rT")
                with nc.allow_low_precision(reason="softmax 1/sum -> fp16"):
                    V.reciprocal(rT[:], Pps[96:108, :])
                sbps = pst.tile([96, BL], F32, tag="small", name="sbps")
                mm(sbps[:], cs["Ebc"][:], rT[:], start=True, stop=True)
                sb96 = wpool.tile([96, BL], F32, tag="sb96", name="sb96")
                V.tensor_copy(sb96[:], sbps[:])
                V.tensor_mul(st["xT"][0:96, :], Pps[0:96, :], sb96[:])
                dbg_tiles = {"lnd": lnd, "diff": es, "u": u_sb,
                             "sb96": sb96, "rT": rT}
                if os.environ.get("KXTLATE"):
                    V.tensor_copy(st["xT"][96:108, :],
                                  t_rssT[:, t * BL:(t + 1) * BL])

                # ---------- two LSTM layers ----------
                for ly in range(2):
                    wih = cs["Wih0T"] if ly == 0 else cs["Wih1T"]
                    xin = st["xT"] if ly == 0 else st["h0"]
                    cst = st["c0"] if ly == 0 else st["c1"]
                    if PREFETCH:
                        gps = gps0 if ly == 0 else gps1
                        gate_mms(gps, wih, xin, False, True)
                    else:
                        whh = cs["Whh0T"] if ly == 0 else cs["Whh1T"]
                        hin = st["h0"] if ly == 0 else st["h1w"][:, 0:BL]
                        gps = pg.tile([128, 4 * BL], F32, tag="g",
                                      name="gps")
                        gate_mms(gps, whh, hin, True, False)
                        gate_mms(gps, wih, xin, False, True)

                    # i,f,g sigmoids feed the cell update; o's sigmoid is
                    # computed while exp(-2c) sits on the ACT engine.
                    e0 = wpool.tile([128, 3 * BL], F32, tag=f"e{ly}",
                                    name="e0t")
                    act(e0[:], gps[:, 0:3 * BL], AFT.Exp, scale=-1.0)
                    eo = wpool.tile([128, BL], F32, tag=f"eo{ly}", name="eot")
                    act(eo[:], gps[:, 3 * BL:4 * BL], AFT.Exp, scale=-1.0)
                    ea = wpool.tile([128, 3 * BL], F32, tag=f"ea{ly}",
                                    name="eat")
                    V.tensor_scalar_add(ea[:], e0[:], 1.0)
                    rg = wpool.tile([128, 3 * BL], F32, tag=f"rg{ly}",
                                    name="rgt")
                    V.reciprocal_approx_fast(rg[:], ea[:])
                    tg = wpool.tile([128, BL], F32, tag=f"tg{ly}", name="tgt")
                    V.tensor_scalar(tg[:], rg[:, 2 * BL:3 * BL], 2.0, 1.0,
                                    op0=ALU.mult, op1=ALU.subtract)
                    p_ = wpool.tile([128, BL], F32, tag=f"p{ly}", name="p_t")
                    V.tensor_mul(p_[:], rg[:, BL:2 * BL], cst[:])  # sig(f)*c
                    q_ = wpool.tile([128, BL], F32, tag=f"q{ly}", name="q_t")
                    V.tensor_mul(q_[:], rg[:, 0:BL], tg[:])        # sig(i)*tg
                    V.tensor_add(cst[:], p_[:], q_[:])

                    ec = wpool.tile([128, BL], F32, tag=f"ec{ly}", name="ect")
                    act(ec[:], cst[:], AFT.Exp, scale=-2.0)
                    # o-gate sigmoid on DVE during the exp(-2c) latency
                    eao = wpool.tile([128, BL], F32, tag=f"eao{ly}",
                                     name="eaot")
                    V.tensor_scalar_add(eao[:], eo[:], 1.0)
                    ro = wpool.tile([128, BL], F32, tag=f"ro{ly}", name="rot")
                    V.reciprocal_approx_fast(ro[:], eao[:])
                    eac = wpool.tile([128, BL], F32, tag=f"eac{ly}",
                                     name="eact")
                    V.tensor_scalar_add(eac[:], ec[:], 1.0)
                    rc = wpool.tile([128, BL], F32, tag=f"rc{ly}", name="rct")
                    V.reciprocal_approx_fast(rc[:], eac[:])
                    thc = wpool.tile([128, BL], F32, tag=f"thc{ly}",
                                     name="thct")
                    V.tensor_scalar(thc[:], rc[:], 2.0, 1.0,
                                    op0=ALU.mult, op1=ALU.subtract)
                    hout = st["h0"][:] if ly == 0 else st["h1w"][:, 0:BL]
                    V.tensor_mul(hout, ro[:], thc[:])

                    # prefetch next step's Whh gates the moment h is out
                    if PREFETCH and ly == 0 and t + 1 < nsteps:
                        g0_next = pg.tile([128, 4 * BL], F32, tag="g",
                                          name="gps")
                        gate_mms(g0_next, cs["Whh0T"], st["h0"], True, False)

                # ---------- LayerNorm + fc head ----------
                # stat split: the sum(h) half issues right at h1 so the
                # -mu/m2/vv/rstd pole starts ~350ns earlier; sum(h^2)
                # follows the h^2 multiply. Emitted BEFORE the whh1
                # prefetch so the in-order PE doesn't queue 4 gate
                # matmuls ahead of them.
                stat = pst.tile([1, 2 * BL], F32, tag="small", name="stat")
                mm(stat[0:1, 0:BL], cs["ones128"][:], st["h1w"][:, 0:BL],
                   start=True, stop=True)
                V.tensor_mul(st["h1w"][:, BL:2 * BL], st["h1w"][:, 0:BL],
                             st["h1w"][:, 0:BL])
                mm(stat[0:1, BL:2 * BL], cs["ones128"][:],
                   st["h1w"][:, BL:2 * BL], start=True, stop=True)
                if PREFETCH and t + 1 < nsteps:
                    g1_next = pg.tile([128, 4 * BL], F32, tag="g",
                                      name="gps")
                    gate_mms(g1_next, cs["Whh1T"], st["h1w"][:, 0:BL],
                             True, False)

                bsrc = lnpool.tile([1, 2 * BL], LOWP, tag="bsrc", name="bsrc")
                V.tensor_scalar_mul(bsrc[0:1, 0:BL], stat[0:1, 0:BL],
                                    -1.0 / H)  # -mu (fp16 is plenty)
                m2 = lnpool.tile([1, BL], F32, tag="m2", name="m2")
                V.tensor_mul(m2[:], bsrc[0:1, 0:BL], bsrc[0:1, 0:BL])
                vv = lnpool.tile([1, BL], F32, tag="vv", name="vv")
                V.scalar_tensor_tensor(vv[:], stat[0:1, BL:2 * BL], 1.0 / H,
                                       m2[:], op0=ALU.mult, op1=ALU.subtract)
                lv = lnpool.tile([1, BL], F32, tag="lv", name="lv")
                act(lv[:], vv[:], AFT.Ln, bias=cs["epsl"][:, 0:1])
                act(bsrc[0:1, BL:2 * BL], lv[:], AFT.Exp, scale=-0.5)

                # relu(rstd*(W1g@(h1-mu))) = rstd*relu(W1g@h1 - mu*w1s):
                # centering is a K=1 accumulating matmul; rstd scales the
                # (bias-free) head output at the very end.
                a2ps = pst.tile([64, BL], F32, tag="small", name="a2ps")
                mm(a2ps[:], cs["W1T"][:], st["h1w"][:, 0:BL],
                   start=True, stop=False)
                mm(a2ps[:], cs["w1s"][:], bsrc[0:1, 0:BL],
                   start=False, stop=True)
                if meta["b1f_zero"]:
                    V.tensor_scalar_max(st["Are"][0:64, :], a2ps[:], 0.0)
                else:
                    act(st["Are"][0:64, :], a2ps[:], AFT.Relu,
                        bias=cs["b1f"][:, 0:1])
                prps = pst.tile([3, BL], F32, tag="small", name="prps")
                mm(prps[:], cs["W2T"][:], st["Are"][:], start=True, stop=True)
                # r3 broadcast reuses the dist PSUM bank (free this late
                # in the step; next dist write waits for r3sb's read)
                if os.environ.get("KR3SEP"):
                    r3ps = pbc.tile([3, BL], F32, tag="bc", name="r3ps")[:]
                else:
                    r3ps = dps[0:3, 0:BL]
                mm(r3ps, cs["ones13"][:], bsrc[0:1, BL:2 * BL],
                   start=True, stop=True)
                r3sb = lnpool.tile([3, BL], F32, tag="r3sb", name="r3sb")
                V.tensor_copy(r3sb[:], r3ps)
                nxt = t_out[:, t * BL:(t + 1) * BL]
                if t + 1 < nsteps:
                    V.tensor_mul(nxt, prps[:], r3sb[:])
                    if os.environ.get("KPREV3CAST"):
                        V.tensor_copy(st["prev3"][:], nxt)
                    else:
                        V.tensor_mul(st["prev3"][:], prps[:], r3sb[:])
                    V.tensor_scalar_add(st["prevaux"][0:3, :], nxt, -CEN)
                    V.tensor_mul(st["prevaux"][32:35, :],
                                 st["prevaux"][0:3, :],
                                 st["prevaux"][0:3, :])
                else:
                    V.tensor_mul(nxt, prps[:], r3sb[:])
                    q1_next = pq1.tile([64, R], F32, tag="q1", name="q1ps")
                    mm(q1_next[:], cs["qW1a"][:], qr_next[:],
                       start=True, stop=False, skip_group_check=True)

            if os.environ.get("KDBG"):
                for nm, ap, shp, dt_ in [
                        ("d_lnd", dbg_tiles["lnd"], [128, NC * BL], F32),
                        ("d_diff", dbg_tiles["diff"], [128, NC * BL], F32),
                        ("d_u", dbg_tiles["u"], [128, NC * BL], LOWP),
                        ("d_sb96", dbg_tiles["sb96"], [96, BL], F32),
                        ("d_rT", dbg_tiles["rT"], [RSS, BL], LOWP),
                        ("d_prevaux", st["prevaux"], [35, BL], LOWP),
                        ("d_q1re", st["q1re"], [65, R], LOWP),
                        ("d_xT", st["xT"], [109, BL], LOWP),
                        ("d_h0", st["h0"], [H, BL], LOWP),
                        ("d_c0", st["c0"], [H, BL], F32),
                        ("d_h1w", st["h1w"], [H, 2 * BL], LOWP),
                        ("d_Are", st["Are"], [65, BL], LOWP),
                        ("d_prev3", st["prev3"], [3, BL], LOWP)]:
                    dd = nc.dram_tensor(nm, shp, dt_,
                                        kind="ExternalOutput").ap()
                    nc.sync.dma_start(dd[:], ap[:])
            nc.sync.dma_start(d_out[:], t_out[:])

    nc.compile()
    return nc


def make_in_maps(consts, init, rss_seq):
    base = {k: v for k, (v, _) in {**consts, **init}.items()}
    in_maps = []
    for k in range(NCORES):
        rss_q, rssT = _per_core_rss(rss_seq[k * BL:(k + 1) * BL])
        m = dict(base)
        m["rss_q"] = rss_q
        m["rssT"] = rssT
        in_maps.append(m)
    return in_maps


def kernel(**inputs):
    rss_seq = _f32(inputs["rss_seq"])
    consts, init, meta = _host_prep(inputs)
    nc = build_nc(consts, init, meta, nsteps=T)
    in_maps = make_in_maps(consts, init, rss_seq)
    res = run_bass_kernel_spmd(nc, in_maps, core_ids=list(range(NCORES)))
    outs = []
    for k in range(NCORES):
        o = res.results[k]["out"]
        outs.append(np.asarray(o).reshape(3, T, BL).transpose(2, 1, 0))
    return np.ascontiguousarray(np.concatenate(outs, 0))
